# revision 6
# baseline (speedup 1.0000x reference)
"""Trainium2 Bass kernel for nn_EntropicOTQuantileRegression.

Math (reference):
    hX = X @ W0[:64]; hY = Y @ W0[64:]
    h  = sp(hX[i] + hY[j] + b0); h = sp(h@W1+b1); h = sp(h@W2+b2)
    psi[i,j] = (h @ Wout + bout); cost = U @ Y.T
    out[i] = eps*logmeanexp((cost[i,:]-psi[i,:])/eps)        (eps=0.1)

Sharding: 8 cores, data parallel over rows of X (128 rows each); Y/U-rows/
weights replicated or sharded accordingly; no collectives.

On-core layout: hidden dim (128) on partitions, pair columns on the free
axis. Per group of 4 local i's (4096 pair-columns):
    pre0 = hYT_b0 + hXT[:,i]          (DVE, per-i broadcast add)
    a0   = Ln(Exp(pre0)+1)            (ACT; softplus = exp+ln, no native op)
    l1   = W1.T @ a0 -> PSUM          (PE, fp32)
    a1   = Ln(Exp(l1 + b1)+1)         (ACT)
    l2, a2 likewise
    psi  = a2-chunk.T @ Wout          (PE transpose-trick: pairs on psum
                                       partitions, 128 pairs per matmul)
Tail: transpose psi back to [i, j] layout via PE transpose, then
rowmax/exp/sum/ln for the stabilized logmeanexp. bout folded in on host.

Constraint honored throughout: this toolchain's walrus accepts at most ONE
semaphore wait per compute instruction, so the pipeline is a strict
PE<->ACT ping-pong (DVE only feeds ACT / reads PE via already-waited
ticks), and the framework's 16-wait kernel-tail drain is monkeypatched
into a chain of single-wait drains.
"""
import math

import numpy as np

N = 1024
F = 64
R = 8
H = 128
NCORES = 8
NLOC = N // NCORES          # 128 local i rows per core
AB = 4                      # i's per act0 group
NG = NLOC // AB             # 32 groups (default; override via KERNEL_NG)
EPS = 0.1

_cached = {}


def _patch_drain():
    import concourse.tile as tile
    import concourse.mybir as mybir
    from concourse.vector_clock import ScopedClock

    if getattr(tile.TileContext, "_ant_drain_patched", False):
        return

    def _drain_and_barrier(self, tick_clock, wait_clock):
        nc = self.nc
        d0 = nc.sync.drain()
        wait_clock.add_sem_waits(
            d0.ins, ScopedClock({None: tick_clock.global_clock})
        )
        si = d0.ins.sync_info
        if si is not None and si.on_wait and len(si.on_wait) > 1:
            rest = list(si.on_wait[1:])
            d0.ins.sync_info = mybir.SyncInfo(
                on_wait=[si.on_wait[0]], on_update=list(si.on_update or [])
            )
            for w in rest:
                d = nc.sync.drain()
                d.ins.sync_info = mybir.SyncInfo(on_wait=[w], on_update=[])
        nc.all_engine_barrier()
        assert self.sems is not None
        popped = nc._tile_sem_poison_stack.pop()
        assert popped is self._sem_poison
        nc.clear_and_free_semaphores(list(self.sems.allocated().values()))
        nc.all_engine_barrier()

    tile.TileContext._drain_and_barrier = _drain_and_barrier
    tile.TileContext._ant_drain_patched = True


def _split_waits(nc, mybir):
    """Walrus in this toolchain accepts at most one semaphore wait per
    instruction; hoist extra waits onto injected same-engine NoOps."""
    n = 0
    for f in nc.m.functions:
        for bb in f.blocks:
            out = []
            for inst in bb.instructions:
                si = getattr(inst, "sync_info", None)
                if si is not None and si.on_wait and len(si.on_wait) > 1:
                    waits = list(si.on_wait)
                    for w in waits[:-1]:
                        out.append(mybir.InstNoOp(
                            name=f"antw-{nc.next_id()}",
                            engine=inst.engine,
                            sync_info=mybir.SyncInfo(on_wait=[w], on_update=[]),
                        ))
                    inst.sync_info = mybir.SyncInfo(
                        on_wait=[waits[-1]],
                        on_update=list(si.on_update or []),
                    )
                    n += 1
                out.append(inst)
            bb.instructions = out
    return n


def _build(variant="f32", repeat=1, ng=None):
    import concourse.bass as bass
    import concourse.tile as tile
    from concourse import mybir

    _patch_drain()
    NGL = NG if ng is None else ng

    f32 = mybir.dt.float32
    f32r = mybir.dt.float32r
    mmdt = f32r if variant == "f32r" else f32
    AF = mybir.ActivationFunctionType
    X_ = mybir.AxisListType.X

    nc = bass.Bass()
    dXT = nc.dram_tensor("XT", [F, NLOC], f32, kind="ExternalInput")
    dYT = nc.dram_tensor("YT", [R, N], f32, kind="ExternalInput")
    dUT = nc.dram_tensor("UT", [R, NLOC], f32, kind="ExternalInput")
    dW0F = nc.dram_tensor("W0F", [F, H], f32, kind="ExternalInput")
    dW0R = nc.dram_tensor("W0R", [R, H], f32, kind="ExternalInput")
    dW1 = nc.dram_tensor("W1", [H, H], f32, kind="ExternalInput")
    dW2 = nc.dram_tensor("W2", [H, H], f32, kind="ExternalInput")
    dWO = nc.dram_tensor("WOUT", [H, 2], f32, kind="ExternalInput")
    dB0 = nc.dram_tensor("B0", [H, 1], f32, kind="ExternalInput")
    dB1 = nc.dram_tensor("B1", [H, 1], f32, kind="ExternalInput")
    dB2 = nc.dram_tensor("B2", [H, 1], f32, kind="ExternalInput")
    dID = nc.dram_tensor("IDENT", [H, H], f32, kind="ExternalInput")
    dOUT = nc.dram_tensor("OUT", [NLOC, 1], f32, kind="ExternalOutput")

    with tile.TileContext(nc) as tc, \
         tc.tile_pool(name="sb", bufs=1) as sb, \
         tc.tile_pool(name="wk", bufs=1) as wk, \
         tc.tile_pool(name="ps", bufs=2, space=bass.MemorySpace.PSUM) as ps:

        # ---- loads (SWDGE; no compute deps) ----
        ldXT = sb.tile([F, NLOC], f32, name="ldXT")
        nc.gpsimd.dma_start(ldXT, dXT[:])
        ldYT = sb.tile([R, N], f32, name="ldYT")
        nc.gpsimd.dma_start(ldYT, dYT[:])
        ldUT = sb.tile([R, NLOC], f32, name="ldUT")
        nc.gpsimd.dma_start(ldUT, dUT[:])
        ldW0F = sb.tile([F, H], f32, name="ldW0F")
        nc.gpsimd.dma_start(ldW0F, dW0F[:])
        ldW0R = sb.tile([R, H], f32, name="ldW0R")
        nc.gpsimd.dma_start(ldW0R, dW0R[:])
        ldW1 = sb.tile([H, H], f32, name="ldW1")
        nc.gpsimd.dma_start(ldW1, dW1[:])
        ldW2 = sb.tile([H, H], f32, name="ldW2")
        nc.gpsimd.dma_start(ldW2, dW2[:])
        ldWO = sb.tile([H, 2], f32, name="ldWO")
        nc.gpsimd.dma_start(ldWO, dWO[:])
        ldB0 = sb.tile([H, 1], f32, name="ldB0")
        nc.gpsimd.dma_start(ldB0, dB0[:])
        ldB1 = sb.tile([H, 1], f32, name="ldB1")
        nc.gpsimd.dma_start(ldB1, dB1[:])
        ldB2 = sb.tile([H, 1], f32, name="ldB2")
        nc.gpsimd.dma_start(ldB2, dB2[:])
        ldID = sb.tile([H, H], f32, name="ldID")
        nc.gpsimd.dma_start(ldID, dID[:])

        # ---- DVE staging: every matmul input / ACT bias is DVE-produced so
        # consumers carry exactly one cross-engine wait ----
        sID = sb.tile([H, H], f32, name="sID")
        nc.vector.tensor_copy(sID, ldID)
        sXT = sb.tile([F, NLOC], f32, name="sXT")
        nc.vector.tensor_copy(sXT, ldXT)
        sYT = sb.tile([R, N], f32, name="sYT")
        nc.vector.tensor_copy(sYT, ldYT)
        sUT = sb.tile([R, NLOC], f32, name="sUT")
        nc.vector.tensor_copy(sUT, ldUT)
        sW0F = sb.tile([F, H], f32, name="sW0F")
        nc.vector.tensor_copy(sW0F, ldW0F)
        sW0R = sb.tile([R, H], f32, name="sW0R")
        nc.vector.tensor_copy(sW0R, ldW0R)
        sW1 = sb.tile([H, H], mmdt, name="sW1")
        nc.vector.tensor_copy(sW1, ldW1)
        sW2 = sb.tile([H, H], mmdt, name="sW2")
        nc.vector.tensor_copy(sW2, ldW2)
        sWO = sb.tile([H, 2], mmdt, name="sWO")
        nc.vector.tensor_copy(sWO, ldWO)
        sB0 = sb.tile([H, 1], f32, name="sB0")
        nc.vector.tensor_copy(sB0, ldB0)
        sB1 = sb.tile([H, 1], f32, name="sB1")
        nc.vector.tensor_copy(sB1, ldB1)
        sB2 = sb.tile([H, 1], f32, name="sB2")
        nc.vector.tensor_copy(sB2, ldB2)

        # ---- preamble matmuls; psum readers are ACT (so loop PE waits stay 1)
        # except hXT whose reader is DVE (consumed before any loop matmul) ----
        hXTp = ps.tile([H, NLOC], f32, tag="mm", name="hXTp")
        nc.tensor.matmul(hXTp, sW0F, sXT, start=True, stop=True)
        hXT = sb.tile([H, NLOC], f32, name="hXT")
        nc.vector.tensor_copy(hXT, hXTp)

        # ACT shepherd: absorb the DVE tick for sB0 before Identity uses it.
        shep = sb.tile([H, 1], f32, name="shep")
        nc.scalar.copy(shep, sB0)

        hYp = ps.tile([H, N], f32, tag="mm", name="hYp")
        for k in range(2):
            nc.tensor.matmul(hYp[:, k * 512:(k + 1) * 512], sW0R,
                             sYT[:, k * 512:(k + 1) * 512],
                             start=True, stop=True)
        hYb0 = sb.tile([H, N], f32, name="hYb0")
        nc.scalar.activation(hYb0, hYp, AF.Identity, bias=sB0, scale=1.0)

        costp = ps.tile([H, N], f32, tag="mm", name="costp")
        for k in range(2):
            nc.tensor.matmul(costp[:, k * 512:(k + 1) * 512], sUT,
                             sYT[:, k * 512:(k + 1) * 512],
                             start=True, stop=True)
        cost = sb.tile([H, N], f32, name="cost")
        nc.scalar.copy(cost, costp)

        psiT = sb.tile([H, N], f32, name="psiT")
        psi = sb.tile([H, N], f32, name="psi")

        CW = AB * N  # columns per group

        def preadd(g, tgt):
            for k in range(AB):
                i = g * AB + k
                nc.vector.tensor_scalar_add(
                    tgt[:, k * N:(k + 1) * N], hYb0, hXT[:, i:i + 1])

        pcols = {}

        def emit_mlp(gg, a0t, rp=""):
            e1t = wk.tile([H, CW], f32, tag="e1", bufs=1, name=f"{rp}e1_{gg}")
            for s in range(2):
                l1t = ps.tile([H, 2048], f32, tag="mm", name=f"{rp}l1_{gg}_{s}")
                for k in range(4):
                    nc.tensor.matmul(
                        l1t[:, k * 512:(k + 1) * 512], sW1,
                        a0t[:, s * 2048 + k * 512: s * 2048 + (k + 1) * 512],
                        start=True, stop=True)
                nc.scalar.activation(e1t[:, s * 2048:(s + 1) * 2048], l1t,
                                     AF.Exp, bias=sB1, scale=1.0)
            a1t = wk.tile([H, CW], mmdt, tag="a1", bufs=1, name=f"{rp}a1_{gg}")
            nc.scalar.activation(a1t, e1t, AF.Ln, bias=1.0, scale=1.0)

            e2t = wk.tile([H, CW], f32, tag="e2", bufs=1, name=f"{rp}e2_{gg}")
            for s in range(2):
                l2t = ps.tile([H, 2048], f32, tag="mm", name=f"{rp}l2_{gg}_{s}")
                for k in range(4):
                    nc.tensor.matmul(
                        l2t[:, k * 512:(k + 1) * 512], sW2,
                        a1t[:, s * 2048 + k * 512: s * 2048 + (k + 1) * 512],
                        start=True, stop=True)
                nc.scalar.activation(e2t[:, s * 2048:(s + 1) * 2048], l2t,
                                     AF.Exp, bias=sB2, scale=1.0)
            a2t = wk.tile([H, CW], mmdt, tag="a2", bufs=1, name=f"{rp}a2_{gg}")
            nc.scalar.activation(a2t, e2t, AF.Ln, bias=1.0, scale=1.0)

            nw = 2 if variant == "f32r" else 1
            for s in range(2):
                pc = ps.tile([H, 16 * nw], f32, tag="mm",
                             name=f"{rp}pcol_{gg}_{s}")
                for c in range(16):
                    nc.tensor.matmul(
                        pc[:, c * nw:(c + 1) * nw],
                        a2t[:, s * 2048 + c * H: s * 2048 + (c + 1) * H],
                        sWO[:, 0:nw], start=True, stop=True)
                pcols[(gg, s)] = pc

        def drains(gg, rp=""):
            nw = 2 if variant == "f32r" else 1
            for s in range(2):
                iA = gg * AB + 2 * s
                pc = pcols.pop((gg, s))
                nc.vector.tensor_copy(psiT[:, iA * 8: iA * 8 + 16],
                                      pc[:, 0:16 * nw:nw])

        for rep in range(repeat):
            rp = f"r{rep}_"
            pre0_t = {}
            pre0_t[0] = wk.tile([H, CW], f32, tag="pre0", bufs=2,
                                name=rp + "pre0_0")
            preadd(0, pre0_t[0])
            for g in range(NGL):
                e0t = wk.tile([H, CW], f32, tag="e0", bufs=1,
                              name=f"{rp}e0_{g}")
                nc.scalar.activation(e0t, pre0_t.pop(g), AF.Exp)
                a0t = wk.tile([H, CW], mmdt, tag="a0", bufs=2,
                              name=f"{rp}a0_{g}")
                nc.scalar.activation(a0t, e0t, AF.Ln, bias=1.0, scale=1.0)
                if g >= 2:
                    drains(g - 2, rp)
                if g + 1 < NGL:
                    pre0_t[g + 1] = wk.tile([H, CW], f32, tag="pre0", bufs=2,
                                            name=f"{rp}pre0_{g + 1}")
                    preadd(g + 1, pre0_t[g + 1])
                if g >= 1:
                    emit_mlp(g - 1, a0_prev, rp)
                a0_prev = a0t

            drains(NGL - 2, rp)
            emit_mlp(NGL - 1, a0_prev, rp)
            drains(NGL - 1, rp)

        # ---- tail: psi (i-major) from psiT via PE transposes ----
        for jc in range(8):
            ptt = ps.tile([H, H], f32, tag="mm", name=f"pt_{jc}")
            nc.tensor.transpose(ptt, psiT[:, jc:N:8], sID)
            nc.vector.tensor_copy(psi[:, jc * H:(jc + 1) * H], ptt)

        slack = sb.tile([H, N], f32, name="slack")
        nc.vector.tensor_sub(slack, cost, psi)
        rowmax = sb.tile([H, 1], f32, name="rowmax")
        nc.vector.reduce_max(rowmax, slack, axis=X_)
        negb = sb.tile([H, 1], f32, name="negb")
        nc.vector.tensor_scalar_mul(negb, rowmax, -10.0)

        et = sb.tile([H, N], f32, name="et")
        rowsum = sb.tile([H, 1], f32, name="rowsum")
        nc.scalar.activation(et, slack, AF.Exp, bias=negb, scale=10.0,
                             accum_out=rowsum)
        lns = sb.tile([H, 1], f32, name="lns")
        nc.scalar.activation(lns, rowsum, AF.Ln)

        v1 = sb.tile([H, 1], f32, name="v1")
        nc.vector.tensor_scalar_mul(v1, lns, EPS)
        v2 = sb.tile([H, 1], f32, name="v2")
        nc.vector.tensor_scalar_add(v2, v1, -EPS * math.log(float(N)))
        psie = sb.tile([H, 1], f32, name="psie")
        nc.vector.tensor_add(psie, v2, rowmax)

        nc.sync.dma_start(dOUT[:], psie)

    _split_waits(nc, mybir)
    return nc


def _get_nc():
    import os
    variant = os.environ.get("KERNEL_VARIANT", "f32")
    repeat = int(os.environ.get("KERNEL_REPEAT", "1"))
    ng = os.environ.get("KERNEL_NG")
    ng = int(ng) if ng else None
    key = ("nc", variant, repeat, ng)
    if key not in _cached:
        _cached[key] = _build(variant, repeat, ng)
    return _cached[key]


def kernel(**inputs):
    from concourse.bass_utils import run_bass_kernel_spmd

    X = np.asarray(inputs["X"], np.float32)
    U = np.asarray(inputs["U"], np.float32)
    Y = np.asarray(inputs["Y"], np.float32)
    W0 = np.asarray(inputs["W0"], np.float32)
    b0 = np.asarray(inputs["b0"], np.float32)
    W1 = np.asarray(inputs["W1"], np.float32)
    b1 = np.asarray(inputs["b1"], np.float32)
    W2 = np.asarray(inputs["W2"], np.float32)
    b2 = np.asarray(inputs["b2"], np.float32)
    Wout = np.asarray(inputs["Wout"], np.float32)
    bout = np.asarray(inputs["bout"], np.float32)

    YT = np.ascontiguousarray(Y.T)
    shared = {
        "YT": YT,
        "W0F": np.ascontiguousarray(W0[:F]),
        "W0R": np.ascontiguousarray(W0[F:]),
        "W1": np.ascontiguousarray(W1),
        "W2": np.ascontiguousarray(W2),
        "WOUT": np.ascontiguousarray(np.repeat(Wout, 2, axis=1)),
        "B0": np.ascontiguousarray(b0[:, None]),
        "B1": np.ascontiguousarray(b1[:, None]),
        "B2": np.ascontiguousarray(b2[:, None]),
        "IDENT": np.eye(H, dtype=np.float32),
    }
    in_maps = []
    for c in range(NCORES):
        sl = slice(c * NLOC, (c + 1) * NLOC)
        m = dict(shared)
        m["XT"] = np.ascontiguousarray(X[sl].T)
        m["UT"] = np.ascontiguousarray(U[sl].T)
        in_maps.append(m)

    nc = _get_nc()
    res = run_bass_kernel_spmd(nc, in_maps, core_ids=list(range(NCORES)))
    _cached["last_res"] = res
    out = np.concatenate([res.results[c]["OUT"] for c in range(NCORES)], axis=0)
    return (out - bout[0]).astype(np.float32)


# revision 9
# speedup vs baseline: 1.3284x; 1.3284x over previous
"""Trainium2 Bass kernel for nn_EntropicOTQuantileRegression.

Math (reference):
    hX = X @ W0[:64]; hY = Y @ W0[64:]
    h  = sp(hX[i] + hY[j] + b0); h = sp(h@W1+b1); h = sp(h@W2+b2)
    psi[i,j] = (h @ Wout + bout); cost = U @ Y.T
    out[i] = eps*logmeanexp((cost[i,:]-psi[i,:])/eps)        (eps=0.1)

Sharding: 8 cores, data parallel over rows of X (128 rows each); Y/U-rows/
weights replicated or sharded accordingly; no collectives.

On-core layout: hidden dim (128) on partitions, pair columns on the free
axis. Per group of 4 local i's (4096 pair-columns):
    pre0 = hYT_b0 + hXT[:,i]          (DVE, per-i broadcast add)
    a0   = Ln(Exp(pre0)+1)            (ACT; softplus = exp+ln, no native op)
    l1   = W1.T @ a0 -> PSUM          (PE, fp32)
    a1   = Ln(Exp(l1 + b1)+1)         (ACT)
    l2, a2 likewise
    psi  = a2-chunk.T @ Wout          (PE transpose-trick: pairs on psum
                                       partitions, 128 pairs per matmul)
Tail: transpose psi back to [i, j] layout via PE transpose, then
rowmax/exp/sum/ln for the stabilized logmeanexp. bout folded in on host.

Constraint honored throughout: this toolchain's walrus accepts at most ONE
semaphore wait per compute instruction, so the pipeline is a strict
PE<->ACT ping-pong (DVE only feeds ACT / reads PE via already-waited
ticks), and the framework's 16-wait kernel-tail drain is monkeypatched
into a chain of single-wait drains.
"""
import math

import numpy as np

N = 1024
F = 64
R = 8
H = 128
NCORES = 8
NLOC = N // NCORES          # 128 local i rows per core
AB = 4                      # i's per act0 group
NG = NLOC // AB             # 32 groups (default; override via KERNEL_NG)
EPS = 0.1

_cached = {}


def _patch_drain():
    import concourse.tile as tile
    import concourse.mybir as mybir
    from concourse.vector_clock import ScopedClock

    if getattr(tile.TileContext, "_ant_drain_patched", False):
        return

    def _drain_and_barrier(self, tick_clock, wait_clock):
        nc = self.nc
        d0 = nc.sync.drain()
        wait_clock.add_sem_waits(
            d0.ins, ScopedClock({None: tick_clock.global_clock})
        )
        si = d0.ins.sync_info
        if si is not None and si.on_wait and len(si.on_wait) > 1:
            rest = list(si.on_wait[1:])
            d0.ins.sync_info = mybir.SyncInfo(
                on_wait=[si.on_wait[0]], on_update=list(si.on_update or [])
            )
            for w in rest:
                d = nc.sync.drain()
                d.ins.sync_info = mybir.SyncInfo(on_wait=[w], on_update=[])
        nc.all_engine_barrier()
        assert self.sems is not None
        popped = nc._tile_sem_poison_stack.pop()
        assert popped is self._sem_poison
        nc.clear_and_free_semaphores(list(self.sems.allocated().values()))
        nc.all_engine_barrier()

    tile.TileContext._drain_and_barrier = _drain_and_barrier
    tile.TileContext._ant_drain_patched = True


def _split_waits(nc, mybir):
    """Walrus in this toolchain accepts at most one semaphore wait per
    instruction; hoist extra waits onto injected same-engine NoOps."""
    n = 0
    for f in nc.m.functions:
        for bb in f.blocks:
            out = []
            for inst in bb.instructions:
                si = getattr(inst, "sync_info", None)
                if si is not None and si.on_wait and len(si.on_wait) > 1:
                    waits = list(si.on_wait)
                    for w in waits[:-1]:
                        out.append(mybir.InstNoOp(
                            name=f"antw-{nc.next_id()}",
                            engine=inst.engine,
                            sync_info=mybir.SyncInfo(on_wait=[w], on_update=[]),
                        ))
                    inst.sync_info = mybir.SyncInfo(
                        on_wait=[waits[-1]],
                        on_update=list(si.on_update or []),
                    )
                    n += 1
                out.append(inst)
            bb.instructions = out
    return n


def _build(variant="f32", repeat=1, ng=None):
    import concourse.bass as bass
    import concourse.tile as tile
    from concourse import mybir

    _patch_drain()
    NGL = NG if ng is None else ng

    f32 = mybir.dt.float32
    f32r = mybir.dt.float32r
    mmdt = f32r if variant == "f32r" else f32
    AF = mybir.ActivationFunctionType
    X_ = mybir.AxisListType.X

    nc = bass.Bass()
    dXT = nc.dram_tensor("XT", [F, NLOC], f32, kind="ExternalInput")
    dYT = nc.dram_tensor("YT", [R, N], f32, kind="ExternalInput")
    dUT = nc.dram_tensor("UT", [R, NLOC], f32, kind="ExternalInput")
    dW0F = nc.dram_tensor("W0F", [F, H], f32, kind="ExternalInput")
    dW0R = nc.dram_tensor("W0R", [R, H], f32, kind="ExternalInput")
    dW1 = nc.dram_tensor("W1", [H, H], f32, kind="ExternalInput")
    dW2 = nc.dram_tensor("W2", [H, H], f32, kind="ExternalInput")
    dWO = nc.dram_tensor("WOUT", [H, 1], f32, kind="ExternalInput")
    dB0 = nc.dram_tensor("B0", [H, 1], f32, kind="ExternalInput")
    dB1 = nc.dram_tensor("B1", [H, 1], f32, kind="ExternalInput")
    dB2 = nc.dram_tensor("B2", [H, 1], f32, kind="ExternalInput")
    dOUT = nc.dram_tensor("OUT", [NLOC, 1], f32, kind="ExternalOutput")

    with tile.TileContext(nc) as tc, \
         tc.tile_pool(name="sb", bufs=1) as sb, \
         tc.tile_pool(name="wk", bufs=1) as wk, \
         tc.tile_pool(name="ps", bufs=2, space=bass.MemorySpace.PSUM) as ps:

        # ---- loads (SWDGE) ----
        ldXT = sb.tile([F, NLOC], f32, name="ldXT")
        nc.gpsimd.dma_start(ldXT, dXT[:])
        ldYT = sb.tile([R, N], f32, name="ldYT")
        nc.gpsimd.dma_start(ldYT, dYT[:])
        ldUT = sb.tile([R, NLOC], f32, name="ldUT")
        nc.gpsimd.dma_start(ldUT, dUT[:])
        ldW0F = sb.tile([F, H], f32, name="ldW0F")
        nc.gpsimd.dma_start(ldW0F, dW0F[:])
        ldW0R = sb.tile([R, H], f32, name="ldW0R")
        nc.gpsimd.dma_start(ldW0R, dW0R[:])
        ldW1 = sb.tile([H, H], f32, name="ldW1")
        nc.gpsimd.dma_start(ldW1, dW1[:])
        ldW2 = sb.tile([H, H], f32, name="ldW2")
        nc.gpsimd.dma_start(ldW2, dW2[:])
        ldWO = sb.tile([H, 1], f32, name="ldWO")
        nc.gpsimd.dma_start(ldWO, dWO[:])
        ldB0 = sb.tile([H, 1], f32, name="ldB0")
        nc.gpsimd.dma_start(ldB0, dB0[:])
        ldB1 = sb.tile([H, 1], f32, name="ldB1")
        nc.gpsimd.dma_start(ldB1, dB1[:])
        ldB2 = sb.tile([H, 1], f32, name="ldB2")
        nc.gpsimd.dma_start(ldB2, dB2[:])

        # f32r roundings for the big matmuls (DVE converts on copy)
        sW1 = sb.tile([H, H], mmdt, name="sW1")
        nc.vector.tensor_copy(sW1, ldW1)
        sW2 = sb.tile([H, H], mmdt, name="sW2")
        nc.vector.tensor_copy(sW2, ldW2)
        sWO = sb.tile([H, 1], mmdt, name="sWO")
        nc.vector.tensor_copy(sWO, ldWO)

        # ---- preamble (plain f32 matmuls; small) ----
        hXTp = ps.tile([H, NLOC], f32, tag="mm", name="hXTp")
        nc.tensor.matmul(hXTp, ldW0F, ldXT, start=True, stop=True)
        hXT = sb.tile([H, NLOC], f32, name="hXT")
        nc.vector.tensor_copy(hXT, hXTp)

        hYp = ps.tile([H, N], f32, tag="mm", name="hYp")
        for k in range(2):
            nc.tensor.matmul(hYp[:, k * 512:(k + 1) * 512], ldW0R,
                             ldYT[:, k * 512:(k + 1) * 512],
                             start=True, stop=True)
        hYb0 = sb.tile([H, N], f32, name="hYb0")
        nc.scalar.activation(hYb0, hYp, AF.Identity, bias=ldB0, scale=1.0)

        costp = ps.tile([H, N], f32, tag="mm", name="costp")
        for k in range(2):
            nc.tensor.matmul(costp[:, k * 512:(k + 1) * 512], ldUT,
                             ldYT[:, k * 512:(k + 1) * 512],
                             start=True, stop=True)
        cost = sb.tile([H, N], f32, name="cost")
        nc.scalar.copy(cost, costp)

        psi = sb.tile([H, N], f32, name="psi")

        CW = AB * N  # columns per group

        def preadd(g, tgt):
            for k in range(AB):
                i = g * AB + k
                nc.vector.tensor_scalar_add(
                    tgt[:, k * N:(k + 1) * N], hYb0, hXT[:, i:i + 1])

        pcols = {}
        a2s = {}

        def mlp_l12(gg, a0t, rp=""):
            e1t = wk.tile([H, CW], f32, tag="e1", bufs=1, name=f"{rp}e1_{gg}")
            for s in range(2):
                l1t = ps.tile([H, 2048], f32, tag="mm", name=f"{rp}l1_{gg}_{s}")
                for k in range(4):
                    nc.tensor.matmul(
                        l1t[:, k * 512:(k + 1) * 512], sW1,
                        a0t[:, s * 2048 + k * 512: s * 2048 + (k + 1) * 512],
                        start=True, stop=True)
                nc.scalar.activation(e1t[:, s * 2048:(s + 1) * 2048], l1t,
                                     AF.Exp, bias=ldB1, scale=1.0)
            a1t = wk.tile([H, CW], mmdt, tag="a1", bufs=1, name=f"{rp}a1_{gg}")
            nc.scalar.activation(a1t, e1t, AF.Ln, bias=1.0, scale=1.0)

            e2t = wk.tile([H, CW], f32, tag="e2", bufs=1, name=f"{rp}e2_{gg}")
            for s in range(2):
                l2t = ps.tile([H, 2048], f32, tag="mm", name=f"{rp}l2_{gg}_{s}")
                for k in range(4):
                    nc.tensor.matmul(
                        l2t[:, k * 512:(k + 1) * 512], sW2,
                        a1t[:, s * 2048 + k * 512: s * 2048 + (k + 1) * 512],
                        start=True, stop=True)
                nc.scalar.activation(e2t[:, s * 2048:(s + 1) * 2048], l2t,
                                     AF.Exp, bias=ldB2, scale=1.0)
            a2t = wk.tile([H, CW], mmdt, tag="a2", bufs=2, name=f"{rp}a2_{gg}")
            nc.scalar.activation(a2t, e2t, AF.Ln, bias=1.0, scale=1.0)
            a2s[gg] = a2t

        def psi_mms(gg, rp=""):
            a2t = a2s.pop(gg)
            for s in range(2):
                pr = ps.tile([1, 2048], f32, tag="mm",
                             name=f"{rp}psirow_{gg}_{s}")
                for k in range(4):
                    nc.tensor.matmul(
                        pr[0:1, k * 512:(k + 1) * 512], sWO[:, 0:1],
                        a2t[:, s * 2048 + k * 512: s * 2048 + (k + 1) * 512],
                        start=True, stop=True)
                pcols[(gg, s)] = pr

        def drains(gg, rp=""):
            for s in range(2):
                iA = gg * AB + 2 * s
                pr = pcols.pop((gg, s))
                row = wk.tile([1, 2048], f32, tag="rows", bufs=2,
                              name=f"{rp}row_{gg}_{s}")
                nc.vector.tensor_copy(row, pr)
                nc.sync.dma_start(psi[iA:iA + 1, :], row[0:1, 0:1024])
                nc.sync.dma_start(psi[iA + 1:iA + 2, :], row[0:1, 1024:2048])

        for rep in range(repeat):
            rp = f"r{rep}_"
            pre0_t = {}
            pre0_t[0] = wk.tile([H, CW], f32, tag="pre0", bufs=2,
                                name=rp + "pre0_0")
            preadd(0, pre0_t[0])
            for g in range(NGL):
                p0 = pre0_t.pop(g)
                nc.scalar.activation(p0, p0, AF.Exp)  # in place
                a0t = wk.tile([H, CW], mmdt, tag="a0", bufs=2,
                              name=f"{rp}a0_{g}")
                nc.scalar.activation(a0t, p0, AF.Ln, bias=1.0, scale=1.0)
                if g >= 3:
                    drains(g - 3, rp)
                if g + 1 < NGL:
                    pre0_t[g + 1] = wk.tile([H, CW], f32, tag="pre0", bufs=2,
                                            name=f"{rp}pre0_{g + 1}")
                    preadd(g + 1, pre0_t[g + 1])
                if g >= 1:
                    mlp_l12(g - 1, a0_prev, rp)
                if g >= 2:
                    psi_mms(g - 2, rp)
                a0_prev = a0t

            mlp_l12(NGL - 1, a0_prev, rp)
            psi_mms(NGL - 2, rp)
            psi_mms(NGL - 1, rp)
            drains(NGL - 3, rp)
            drains(NGL - 2, rp)
            drains(NGL - 1, rp)

        # ---- tail: stabilized logmeanexp over j ----
        slack = sb.tile([H, N], f32, name="slack")
        nc.vector.tensor_sub(slack, cost, psi)
        rowmax = sb.tile([H, 1], f32, name="rowmax")
        nc.vector.reduce_max(rowmax, slack, axis=X_)
        negb = sb.tile([H, 1], f32, name="negb")
        nc.vector.tensor_scalar_mul(negb, rowmax, -10.0)

        et = sb.tile([H, N], f32, name="et")
        rowsum = sb.tile([H, 1], f32, name="rowsum")
        nc.scalar.activation(et, slack, AF.Exp, bias=negb, scale=10.0,
                             accum_out=rowsum)
        lns = sb.tile([H, 1], f32, name="lns")
        nc.scalar.activation(lns, rowsum, AF.Ln)

        v1 = sb.tile([H, 1], f32, name="v1")
        nc.vector.tensor_scalar_mul(v1, lns, EPS)
        v2 = sb.tile([H, 1], f32, name="v2")
        nc.vector.tensor_scalar_add(v2, v1, -EPS * math.log(float(N)))
        psie = sb.tile([H, 1], f32, name="psie")
        nc.vector.tensor_add(psie, v2, rowmax)

        nc.sync.dma_start(dOUT[:], psie)

    _split_waits(nc, mybir)
    return nc


def _get_nc():
    import os
    variant = os.environ.get("KERNEL_VARIANT", "f32")
    repeat = int(os.environ.get("KERNEL_REPEAT", "1"))
    ng = os.environ.get("KERNEL_NG")
    ng = int(ng) if ng else None
    key = ("nc", variant, repeat, ng)
    if key not in _cached:
        _cached[key] = _build(variant, repeat, ng)
    return _cached[key]


def kernel(**inputs):
    from concourse.bass_utils import run_bass_kernel_spmd

    X = np.asarray(inputs["X"], np.float32)
    U = np.asarray(inputs["U"], np.float32)
    Y = np.asarray(inputs["Y"], np.float32)
    W0 = np.asarray(inputs["W0"], np.float32)
    b0 = np.asarray(inputs["b0"], np.float32)
    W1 = np.asarray(inputs["W1"], np.float32)
    b1 = np.asarray(inputs["b1"], np.float32)
    W2 = np.asarray(inputs["W2"], np.float32)
    b2 = np.asarray(inputs["b2"], np.float32)
    Wout = np.asarray(inputs["Wout"], np.float32)
    bout = np.asarray(inputs["bout"], np.float32)

    YT = np.ascontiguousarray(Y.T)
    shared = {
        "YT": YT,
        "W0F": np.ascontiguousarray(W0[:F]),
        "W0R": np.ascontiguousarray(W0[F:]),
        "W1": np.ascontiguousarray(W1),
        "W2": np.ascontiguousarray(W2),
        "WOUT": np.ascontiguousarray(Wout),
        "B0": np.ascontiguousarray(b0[:, None]),
        "B1": np.ascontiguousarray(b1[:, None]),
        "B2": np.ascontiguousarray(b2[:, None]),
    }
    in_maps = []
    for c in range(NCORES):
        sl = slice(c * NLOC, (c + 1) * NLOC)
        m = dict(shared)
        m["XT"] = np.ascontiguousarray(X[sl].T)
        m["UT"] = np.ascontiguousarray(U[sl].T)
        in_maps.append(m)

    nc = _get_nc()
    res = run_bass_kernel_spmd(nc, in_maps, core_ids=list(range(NCORES)))
    _cached["last_res"] = res
    out = np.concatenate([res.results[c]["OUT"] for c in range(NCORES)], axis=0)
    return (out - bout[0]).astype(np.float32)


# revision 10
# speedup vs baseline: 1.3284x; 1.0001x over previous
"""Trainium2 Bass kernel for nn_EntropicOTQuantileRegression.

Math (reference):
    hX = X @ W0[:64]; hY = Y @ W0[64:]
    h  = sp(hX[i] + hY[j] + b0); h = sp(h@W1+b1); h = sp(h@W2+b2)
    psi[i,j] = (h @ Wout + bout); cost = U @ Y.T
    out[i] = eps*logmeanexp((cost[i,:]-psi[i,:])/eps)        (eps=0.1)

Sharding: 8 cores, data parallel over rows of X (128 rows each); Y/U-rows/
weights replicated or sharded accordingly; no collectives.

On-core layout: hidden dim (128) on partitions, pair columns on the free
axis. Per group of 4 local i's (4096 pair-columns):
    pre0 = hYT_b0 + hXT[:,i]          (DVE, per-i broadcast add)
    a0   = Ln(Exp(pre0)+1)            (ACT; softplus = exp+ln, no native op)
    l1   = W1.T @ a0 -> PSUM          (PE, fp32)
    a1   = Ln(Exp(l1 + b1)+1)         (ACT)
    l2, a2 likewise
    psi  = a2-chunk.T @ Wout          (PE transpose-trick: pairs on psum
                                       partitions, 128 pairs per matmul)
Tail: transpose psi back to [i, j] layout via PE transpose, then
rowmax/exp/sum/ln for the stabilized logmeanexp. bout folded in on host.

Constraint honored throughout: this toolchain's walrus accepts at most ONE
semaphore wait per compute instruction, so the pipeline is a strict
PE<->ACT ping-pong (DVE only feeds ACT / reads PE via already-waited
ticks), and the framework's 16-wait kernel-tail drain is monkeypatched
into a chain of single-wait drains.
"""
import math

import numpy as np

N = 1024
F = 64
R = 8
H = 128
NCORES = 8
NLOC = N // NCORES          # 128 local i rows per core
AB = 4                      # i's per act0 group
NG = NLOC // AB             # 32 groups (default; override via KERNEL_NG)
EPS = 0.1

_cached = {}


def _patch_drain():
    import concourse.tile as tile
    import concourse.mybir as mybir
    from concourse.vector_clock import ScopedClock

    if getattr(tile.TileContext, "_ant_drain_patched", False):
        return

    def _drain_and_barrier(self, tick_clock, wait_clock):
        nc = self.nc
        d0 = nc.sync.drain()
        wait_clock.add_sem_waits(
            d0.ins, ScopedClock({None: tick_clock.global_clock})
        )
        si = d0.ins.sync_info
        if si is not None and si.on_wait and len(si.on_wait) > 1:
            rest = list(si.on_wait[1:])
            d0.ins.sync_info = mybir.SyncInfo(
                on_wait=[si.on_wait[0]], on_update=list(si.on_update or [])
            )
            for w in rest:
                d = nc.sync.drain()
                d.ins.sync_info = mybir.SyncInfo(on_wait=[w], on_update=[])
        nc.all_engine_barrier()
        assert self.sems is not None
        popped = nc._tile_sem_poison_stack.pop()
        assert popped is self._sem_poison
        nc.clear_and_free_semaphores(list(self.sems.allocated().values()))
        nc.all_engine_barrier()

    tile.TileContext._drain_and_barrier = _drain_and_barrier
    tile.TileContext._ant_drain_patched = True


def _split_waits(nc, mybir):
    """Walrus in this toolchain accepts at most one semaphore wait per
    instruction; hoist extra waits onto injected same-engine NoOps."""
    n = 0
    for f in nc.m.functions:
        for bb in f.blocks:
            out = []
            for inst in bb.instructions:
                si = getattr(inst, "sync_info", None)
                if si is not None and si.on_wait and len(si.on_wait) > 1:
                    waits = list(si.on_wait)
                    for w in waits[:-1]:
                        out.append(mybir.InstNoOp(
                            name=f"antw-{nc.next_id()}",
                            engine=inst.engine,
                            sync_info=mybir.SyncInfo(on_wait=[w], on_update=[]),
                        ))
                    inst.sync_info = mybir.SyncInfo(
                        on_wait=[waits[-1]],
                        on_update=list(si.on_update or []),
                    )
                    n += 1
                out.append(inst)
            bb.instructions = out
    return n


def _build(variant="f32", repeat=1, ng=None):
    import concourse.bass as bass
    import concourse.tile as tile
    from concourse import mybir

    _patch_drain()
    NGL = NG if ng is None else ng

    f32 = mybir.dt.float32
    f32r = mybir.dt.float32r
    mmdt = f32r if variant == "f32r" else f32
    AF = mybir.ActivationFunctionType
    X_ = mybir.AxisListType.X

    nc = bass.Bass()
    dXT = nc.dram_tensor("XT", [F, NLOC], f32, kind="ExternalInput")
    dYT = nc.dram_tensor("YT", [R, N], f32, kind="ExternalInput")
    dUT = nc.dram_tensor("UT", [R, NLOC], f32, kind="ExternalInput")
    dW0F = nc.dram_tensor("W0F", [F, H], f32, kind="ExternalInput")
    dW0R = nc.dram_tensor("W0R", [R, H], f32, kind="ExternalInput")
    dW1 = nc.dram_tensor("W1", [H, H], f32, kind="ExternalInput")
    dW2 = nc.dram_tensor("W2", [H, H], f32, kind="ExternalInput")
    dWO = nc.dram_tensor("WOUT", [H, 1], f32, kind="ExternalInput")
    dB0 = nc.dram_tensor("B0", [H, 1], f32, kind="ExternalInput")
    dB1 = nc.dram_tensor("B1", [H, 1], f32, kind="ExternalInput")
    dB2 = nc.dram_tensor("B2", [H, 1], f32, kind="ExternalInput")
    dOUT = nc.dram_tensor("OUT", [NLOC, 1], f32, kind="ExternalOutput")

    with tile.TileContext(nc) as tc, \
         tc.tile_pool(name="sb", bufs=1) as sb, \
         tc.tile_pool(name="wk", bufs=1) as wk, \
         tc.tile_pool(name="ps", bufs=2, space=bass.MemorySpace.PSUM) as ps:

        # ---- loads (SWDGE) ----
        ldXT = sb.tile([F, NLOC], f32, name="ldXT")
        nc.gpsimd.dma_start(ldXT, dXT[:])
        ldYT = sb.tile([R, N], f32, name="ldYT")
        nc.gpsimd.dma_start(ldYT, dYT[:])
        ldUT = sb.tile([R, NLOC], f32, name="ldUT")
        nc.gpsimd.dma_start(ldUT, dUT[:])
        ldW0F = sb.tile([F, H], f32, name="ldW0F")
        nc.gpsimd.dma_start(ldW0F, dW0F[:])
        ldW0R = sb.tile([R, H], f32, name="ldW0R")
        nc.gpsimd.dma_start(ldW0R, dW0R[:])
        ldW1 = sb.tile([H, H], f32, name="ldW1")
        nc.gpsimd.dma_start(ldW1, dW1[:])
        ldW2 = sb.tile([H, H], f32, name="ldW2")
        nc.gpsimd.dma_start(ldW2, dW2[:])
        ldWO = sb.tile([H, 1], f32, name="ldWO")
        nc.gpsimd.dma_start(ldWO, dWO[:])
        ldB0 = sb.tile([H, 1], f32, name="ldB0")
        nc.gpsimd.dma_start(ldB0, dB0[:])
        ldB1 = sb.tile([H, 1], f32, name="ldB1")
        nc.gpsimd.dma_start(ldB1, dB1[:])
        ldB2 = sb.tile([H, 1], f32, name="ldB2")
        nc.gpsimd.dma_start(ldB2, dB2[:])

        # f32r roundings for the big matmuls (DVE converts on copy)
        sW1 = sb.tile([H, H], mmdt, name="sW1")
        nc.vector.tensor_copy(sW1, ldW1)
        sW2 = sb.tile([H, H], mmdt, name="sW2")
        nc.vector.tensor_copy(sW2, ldW2)
        sWO = sb.tile([H, 1], mmdt, name="sWO")
        nc.vector.tensor_copy(sWO, ldWO)

        # ---- preamble (plain f32 matmuls; small) ----
        hXTp = ps.tile([H, NLOC], f32, tag="mm", name="hXTp")
        nc.tensor.matmul(hXTp, ldW0F, ldXT, start=True, stop=True)
        hXT = sb.tile([H, NLOC], f32, name="hXT")
        nc.vector.tensor_copy(hXT, hXTp)

        hYp = ps.tile([H, N], f32, tag="mm", name="hYp")
        for k in range(2):
            nc.tensor.matmul(hYp[:, k * 512:(k + 1) * 512], ldW0R,
                             ldYT[:, k * 512:(k + 1) * 512],
                             start=True, stop=True)
        hYb0 = sb.tile([H, N], f32, name="hYb0")
        nc.scalar.activation(hYb0, hYp, AF.Identity, bias=ldB0, scale=1.0)

        costp = ps.tile([H, N], f32, tag="mm", name="costp")
        for k in range(2):
            nc.tensor.matmul(costp[:, k * 512:(k + 1) * 512], ldUT,
                             ldYT[:, k * 512:(k + 1) * 512],
                             start=True, stop=True)
        cost = sb.tile([H, N], f32, name="cost")
        nc.scalar.copy(cost, costp)

        psi = sb.tile([H, N], f32, name="psi")

        CW = AB * N  # columns per group

        def preadd(g, tgt):
            for k in range(AB):
                i = g * AB + k
                nc.vector.tensor_scalar_add(
                    tgt[:, k * N:(k + 1) * N], hYb0, hXT[:, i:i + 1])

        pcols = {}
        a2s = {}

        def mlp_l12(gg, a0t, rp=""):
            e1t = wk.tile([H, CW], f32, tag="e1", bufs=1, name=f"{rp}e1_{gg}")
            for s in range(2):
                l1t = ps.tile([H, 2048], f32, tag="mm", name=f"{rp}l1_{gg}_{s}")
                for k in range(4):
                    nc.tensor.matmul(
                        l1t[:, k * 512:(k + 1) * 512], sW1,
                        a0t[:, s * 2048 + k * 512: s * 2048 + (k + 1) * 512],
                        start=True, stop=True)
                nc.scalar.activation(e1t[:, s * 2048:(s + 1) * 2048], l1t,
                                     AF.Exp, bias=ldB1, scale=1.0)
            a1t = wk.tile([H, CW], mmdt, tag="a1", bufs=1, name=f"{rp}a1_{gg}")
            nc.scalar.activation(a1t, e1t, AF.Ln, bias=1.0, scale=1.0)

            e2t = wk.tile([H, CW], f32, tag="e2", bufs=1, name=f"{rp}e2_{gg}")
            for s in range(2):
                l2t = ps.tile([H, 2048], f32, tag="mm", name=f"{rp}l2_{gg}_{s}")
                for k in range(4):
                    nc.tensor.matmul(
                        l2t[:, k * 512:(k + 1) * 512], sW2,
                        a1t[:, s * 2048 + k * 512: s * 2048 + (k + 1) * 512],
                        start=True, stop=True)
                nc.scalar.activation(e2t[:, s * 2048:(s + 1) * 2048], l2t,
                                     AF.Exp, bias=ldB2, scale=1.0)
            a2t = wk.tile([H, CW], mmdt, tag="a2", bufs=2, name=f"{rp}a2_{gg}")
            nc.scalar.activation(a2t, e2t, AF.Ln, bias=1.0, scale=1.0)
            a2s[gg] = a2t

        def psi_mms(gg, rp=""):
            a2t = a2s.pop(gg)
            for s in range(2):
                pr = ps.tile([1, 2048], f32, tag="mm",
                             name=f"{rp}psirow_{gg}_{s}")
                for k in range(4):
                    nc.tensor.matmul(
                        pr[0:1, k * 512:(k + 1) * 512], sWO[:, 0:1],
                        a2t[:, s * 2048 + k * 512: s * 2048 + (k + 1) * 512],
                        start=True, stop=True)
                pcols[(gg, s)] = pr

        def drains(gg, rp=""):
            for s in range(2):
                iA = gg * AB + 2 * s
                pr = pcols.pop((gg, s))
                row = wk.tile([1, 2048], f32, tag="rows", bufs=2,
                              name=f"{rp}row_{gg}_{s}")
                nc.vector.tensor_copy(row, pr)
                nc.sync.dma_start(psi[iA:iA + 1, :], row[0:1, 0:1024])
                nc.sync.dma_start(psi[iA + 1:iA + 2, :], row[0:1, 1024:2048])

        for rep in range(repeat):
            rp = f"r{rep}_"
            pre0_t = {}
            pre0_t[0] = wk.tile([H, CW], f32, tag="pre0", bufs=2,
                                name=rp + "pre0_0")
            preadd(0, pre0_t[0])
            for g in range(NGL):
                p0 = pre0_t.pop(g)
                nc.scalar.activation(p0, p0, AF.Exp)  # in place
                a0t = wk.tile([H, CW], mmdt, tag="a0", bufs=2,
                              name=f"{rp}a0_{g}")
                nc.scalar.activation(a0t, p0, AF.Ln, bias=1.0, scale=1.0)
                if g >= 3:
                    drains(g - 3, rp)
                if g + 1 < NGL:
                    pre0_t[g + 1] = wk.tile([H, CW], f32, tag="pre0", bufs=2,
                                            name=f"{rp}pre0_{g + 1}")
                    preadd(g + 1, pre0_t[g + 1])
                if g >= 1:
                    mlp_l12(g - 1, a0_prev, rp)
                if g >= 2:
                    psi_mms(g - 2, rp)
                a0_prev = a0t

            mlp_l12(NGL - 1, a0_prev, rp)
            psi_mms(NGL - 2, rp)
            psi_mms(NGL - 1, rp)
            drains(NGL - 3, rp)
            drains(NGL - 2, rp)
            drains(NGL - 1, rp)

        # ---- tail: stabilized logmeanexp over j ----
        slack = sb.tile([H, N], f32, name="slack")
        nc.vector.tensor_sub(slack, cost, psi)
        rowmax = sb.tile([H, 1], f32, name="rowmax")
        nc.vector.reduce_max(rowmax, slack, axis=X_)
        negb = sb.tile([H, 1], f32, name="negb")
        nc.vector.tensor_scalar_mul(negb, rowmax, -10.0)

        et = sb.tile([H, N], f32, name="et")
        rowsum = sb.tile([H, 1], f32, name="rowsum")
        nc.scalar.activation(et, slack, AF.Exp, bias=negb, scale=10.0,
                             accum_out=rowsum)
        lns = sb.tile([H, 1], f32, name="lns")
        nc.scalar.activation(lns, rowsum, AF.Ln)

        v1 = sb.tile([H, 1], f32, name="v1")
        nc.vector.tensor_scalar_mul(v1, lns, EPS)
        v2 = sb.tile([H, 1], f32, name="v2")
        nc.vector.tensor_scalar_add(v2, v1, -EPS * math.log(float(N)))
        psie = sb.tile([H, 1], f32, name="psie")
        nc.vector.tensor_add(psie, v2, rowmax)

        nc.sync.dma_start(dOUT[:], psie)

    _split_waits(nc, mybir)
    return nc


def _get_nc():
    import os
    variant = os.environ.get("KERNEL_VARIANT", "f32r")
    repeat = int(os.environ.get("KERNEL_REPEAT", "1"))
    ng = os.environ.get("KERNEL_NG")
    ng = int(ng) if ng else None
    key = ("nc", variant, repeat, ng)
    if key not in _cached:
        _cached[key] = _build(variant, repeat, ng)
    return _cached[key]


def kernel(**inputs):
    from concourse.bass_utils import run_bass_kernel_spmd

    X = np.asarray(inputs["X"], np.float32)
    U = np.asarray(inputs["U"], np.float32)
    Y = np.asarray(inputs["Y"], np.float32)
    W0 = np.asarray(inputs["W0"], np.float32)
    b0 = np.asarray(inputs["b0"], np.float32)
    W1 = np.asarray(inputs["W1"], np.float32)
    b1 = np.asarray(inputs["b1"], np.float32)
    W2 = np.asarray(inputs["W2"], np.float32)
    b2 = np.asarray(inputs["b2"], np.float32)
    Wout = np.asarray(inputs["Wout"], np.float32)
    bout = np.asarray(inputs["bout"], np.float32)

    YT = np.ascontiguousarray(Y.T)
    shared = {
        "YT": YT,
        "W0F": np.ascontiguousarray(W0[:F]),
        "W0R": np.ascontiguousarray(W0[F:]),
        "W1": np.ascontiguousarray(W1),
        "W2": np.ascontiguousarray(W2),
        "WOUT": np.ascontiguousarray(Wout),
        "B0": np.ascontiguousarray(b0[:, None]),
        "B1": np.ascontiguousarray(b1[:, None]),
        "B2": np.ascontiguousarray(b2[:, None]),
    }
    in_maps = []
    for c in range(NCORES):
        sl = slice(c * NLOC, (c + 1) * NLOC)
        m = dict(shared)
        m["XT"] = np.ascontiguousarray(X[sl].T)
        m["UT"] = np.ascontiguousarray(U[sl].T)
        in_maps.append(m)

    nc = _get_nc()
    res = run_bass_kernel_spmd(nc, in_maps, core_ids=list(range(NCORES)))
    _cached["last_res"] = res
    out = np.concatenate([res.results[c]["OUT"] for c in range(NCORES)], axis=0)
    return (out - bout[0]).astype(np.float32)


# revision 15
# speedup vs baseline: 1.3408x; 1.0093x over previous
"""Trainium2 Bass kernel for nn_EntropicOTQuantileRegression.

Math (reference):
    hX = X @ W0[:64]; hY = Y @ W0[64:]
    h  = sp(hX[i] + hY[j] + b0); h = sp(h@W1+b1); h = sp(h@W2+b2)
    psi[i,j] = (h @ Wout + bout); cost = U @ Y.T
    out[i] = eps*logmeanexp((cost[i,:]-psi[i,:])/eps)        (eps=0.1)

Sharding: 8 cores, data parallel over rows of X (128 rows each); Y/U-rows/
weights replicated or sharded accordingly; no collectives.

On-core layout: hidden dim (128) on partitions, pair columns on the free
axis. Per group of 4 local i's (4096 pair-columns):
    pre0 = hYT_b0 + hXT[:,i]          (DVE, per-i broadcast add)
    a0   = Ln(Exp(pre0)+1)            (ACT; softplus = exp+ln, no native op)
    l1   = W1.T @ a0 -> PSUM          (PE, fp32)
    a1   = Ln(Exp(l1 + b1)+1)         (ACT)
    l2, a2 likewise
    psi  = a2-chunk.T @ Wout          (PE transpose-trick: pairs on psum
                                       partitions, 128 pairs per matmul)
Tail: transpose psi back to [i, j] layout via PE transpose, then
rowmax/exp/sum/ln for the stabilized logmeanexp. bout folded in on host.

Constraint honored throughout: this toolchain's walrus accepts at most ONE
semaphore wait per compute instruction, so the pipeline is a strict
PE<->ACT ping-pong (DVE only feeds ACT / reads PE via already-waited
ticks), and the framework's 16-wait kernel-tail drain is monkeypatched
into a chain of single-wait drains.
"""
import math

import numpy as np

N = 1024
F = 64
R = 8
H = 128
NCORES = 8
NLOC = N // NCORES          # 128 local i rows per core
AB = 4                      # i's per act0 group
NG = NLOC // AB             # 32 groups (default; override via KERNEL_NG)
EPS = 0.1

_cached = {}


def _patch_drain():
    import concourse.tile as tile
    import concourse.mybir as mybir
    from concourse.vector_clock import ScopedClock

    if getattr(tile.TileContext, "_ant_drain_patched", False):
        return

    def _drain_and_barrier(self, tick_clock, wait_clock):
        nc = self.nc
        d0 = nc.sync.drain()
        wait_clock.add_sem_waits(
            d0.ins, ScopedClock({None: tick_clock.global_clock})
        )
        si = d0.ins.sync_info
        if si is not None and si.on_wait and len(si.on_wait) > 1:
            rest = list(si.on_wait[1:])
            d0.ins.sync_info = mybir.SyncInfo(
                on_wait=[si.on_wait[0]], on_update=list(si.on_update or [])
            )
            for w in rest:
                d = nc.sync.drain()
                d.ins.sync_info = mybir.SyncInfo(on_wait=[w], on_update=[])
        nc.all_engine_barrier()
        assert self.sems is not None
        popped = nc._tile_sem_poison_stack.pop()
        assert popped is self._sem_poison
        nc.clear_and_free_semaphores(list(self.sems.allocated().values()))
        nc.all_engine_barrier()

    tile.TileContext._drain_and_barrier = _drain_and_barrier
    tile.TileContext._ant_drain_patched = True


def _split_waits(nc, mybir):
    """Walrus in this toolchain accepts at most one semaphore wait per
    instruction; hoist extra waits onto injected same-engine NoOps."""
    n = 0
    for f in nc.m.functions:
        for bb in f.blocks:
            out = []
            for inst in bb.instructions:
                si = getattr(inst, "sync_info", None)
                if si is not None and si.on_wait and len(si.on_wait) > 1:
                    waits = list(si.on_wait)
                    for w in waits[:-1]:
                        out.append(mybir.InstNoOp(
                            name=f"antw-{nc.next_id()}",
                            engine=inst.engine,
                            sync_info=mybir.SyncInfo(on_wait=[w], on_update=[]),
                        ))
                    inst.sync_info = mybir.SyncInfo(
                        on_wait=[waits[-1]],
                        on_update=list(si.on_update or []),
                    )
                    n += 1
                out.append(inst)
            bb.instructions = out
    return n


def _build(variant="f32", repeat=1, ng=None):
    import concourse.bass as bass
    import concourse.tile as tile
    from concourse import mybir

    _patch_drain()
    NGL = NG if ng is None else ng

    f32 = mybir.dt.float32
    f32r = mybir.dt.float32r
    mmdt = f32r if variant == "f32r" else f32
    AF = mybir.ActivationFunctionType
    X_ = mybir.AxisListType.X

    nc = bass.Bass()
    dXT = nc.dram_tensor("XT", [F, NLOC], f32, kind="ExternalInput")
    dYT = nc.dram_tensor("YT", [R, N], f32, kind="ExternalInput")
    dUT = nc.dram_tensor("UT", [R, NLOC], f32, kind="ExternalInput")
    dW0F = nc.dram_tensor("W0F", [F, H], f32, kind="ExternalInput")
    dW0R = nc.dram_tensor("W0R", [R, H], f32, kind="ExternalInput")
    dW1 = nc.dram_tensor("W1", [H, H], f32, kind="ExternalInput")
    dW2 = nc.dram_tensor("W2", [H, H], f32, kind="ExternalInput")
    dWO = nc.dram_tensor("WOUT", [H, 1], f32, kind="ExternalInput")
    dB0 = nc.dram_tensor("B0", [H, 1], f32, kind="ExternalInput")
    dB1 = nc.dram_tensor("B1", [H, 1], f32, kind="ExternalInput")
    dB2 = nc.dram_tensor("B2", [H, 1], f32, kind="ExternalInput")
    dOUT = nc.dram_tensor("OUT", [NLOC, 1], f32, kind="ExternalOutput")

    with tile.TileContext(nc) as tc, \
         tc.tile_pool(name="sb", bufs=1) as sb, \
         tc.tile_pool(name="wk", bufs=1) as wk, \
         tc.tile_pool(name="ps", bufs=2, space=bass.MemorySpace.PSUM) as ps:

        # ---- loads (SWDGE) ----
        ldXT = sb.tile([F, NLOC], f32, name="ldXT")
        nc.gpsimd.dma_start(ldXT, dXT[:])
        ldYT = sb.tile([R, N], f32, name="ldYT")
        nc.gpsimd.dma_start(ldYT, dYT[:])
        ldUT = sb.tile([R, NLOC], f32, name="ldUT")
        nc.gpsimd.dma_start(ldUT, dUT[:])
        ldW0F = sb.tile([F, H], f32, name="ldW0F")
        nc.gpsimd.dma_start(ldW0F, dW0F[:])
        ldW0R = sb.tile([R, H], f32, name="ldW0R")
        nc.gpsimd.dma_start(ldW0R, dW0R[:])
        ldW1 = sb.tile([H, H], f32, name="ldW1")
        nc.gpsimd.dma_start(ldW1, dW1[:])
        ldW2 = sb.tile([H, H], f32, name="ldW2")
        nc.gpsimd.dma_start(ldW2, dW2[:])
        ldWO = sb.tile([H, 1], f32, name="ldWO")
        nc.gpsimd.dma_start(ldWO, dWO[:])
        ldB0 = sb.tile([H, 1], f32, name="ldB0")
        nc.gpsimd.dma_start(ldB0, dB0[:])
        ldB1 = sb.tile([H, 1], f32, name="ldB1")
        nc.gpsimd.dma_start(ldB1, dB1[:])
        ldB2 = sb.tile([H, 1], f32, name="ldB2")
        nc.gpsimd.dma_start(ldB2, dB2[:])

        # f32r roundings for the big matmuls (DVE converts on copy)
        sW1 = sb.tile([H, H], mmdt, name="sW1")
        nc.vector.tensor_copy(sW1, ldW1)
        sW2 = sb.tile([H, H], mmdt, name="sW2")
        nc.vector.tensor_copy(sW2, ldW2)
        sWO = sb.tile([H, 1], mmdt, name="sWO")
        nc.vector.tensor_copy(sWO, ldWO)

        # ---- preamble (plain f32 matmuls; small) ----
        hXTp = ps.tile([H, NLOC], f32, tag="mm", name="hXTp")
        nc.tensor.matmul(hXTp, ldW0F, ldXT, start=True, stop=True)
        hXT = sb.tile([H, NLOC], f32, name="hXT")
        nc.vector.tensor_copy(hXT, hXTp)

        hYp = ps.tile([H, N], f32, tag="mm", name="hYp")
        for k in range(2):
            nc.tensor.matmul(hYp[:, k * 512:(k + 1) * 512], ldW0R,
                             ldYT[:, k * 512:(k + 1) * 512],
                             start=True, stop=True)
        hYb0 = sb.tile([H, N], f32, name="hYb0")
        nc.scalar.activation(hYb0, hYp, AF.Identity, bias=ldB0, scale=1.0)

        costp = ps.tile([H, N], f32, tag="mm", name="costp")
        for k in range(2):
            nc.tensor.matmul(costp[:, k * 512:(k + 1) * 512], ldUT,
                             ldYT[:, k * 512:(k + 1) * 512],
                             start=True, stop=True)
        cost = sb.tile([H, N], f32, name="cost")
        nc.scalar.copy(cost, costp)

        psi = sb.tile([H, N], f32, name="psi")

        CW = AB * N  # columns per group

        def preadd(g, tgt):
            for k in range(AB):
                i = g * AB + k
                nc.vector.tensor_scalar_add(
                    tgt[:, k * N:(k + 1) * N], hYb0, hXT[:, i:i + 1])

        pcols = {}
        a2s = {}

        def mlp_l12(gg, a0t, rp=""):
            e1t = wk.tile([H, CW], f32, tag="e1", bufs=1, name=f"{rp}e1_{gg}")
            for s in range(2):
                l1t = ps.tile([H, 2048], f32, tag="mm", name=f"{rp}l1_{gg}_{s}")
                for k in range(4):
                    nc.tensor.matmul(
                        l1t[:, k * 512:(k + 1) * 512], sW1,
                        a0t[:, s * 2048 + k * 512: s * 2048 + (k + 1) * 512],
                        start=True, stop=True)
                nc.scalar.activation(e1t[:, s * 2048:(s + 1) * 2048], l1t,
                                     AF.Exp, bias=ldB1, scale=1.0)
            a1t = wk.tile([H, CW], mmdt, tag="a1", bufs=1, name=f"{rp}a1_{gg}")
            if gg >= NGL - 5:
                # late groups: halve ln1 so l2(s0) unblocks sooner
                for s in range(2):
                    nc.scalar.activation(a1t[:, s * 2048:(s + 1) * 2048],
                                         e1t[:, s * 2048:(s + 1) * 2048],
                                         AF.Ln, bias=1.0, scale=1.0)
            else:
                nc.scalar.activation(a1t, e1t, AF.Ln, bias=1.0, scale=1.0)

            e2t = wk.tile([H, CW], f32, tag="e2", bufs=1, name=f"{rp}e2_{gg}")
            for s in range(2):
                l2t = ps.tile([H, 2048], f32, tag="mm", name=f"{rp}l2_{gg}_{s}")
                for k in range(4):
                    nc.tensor.matmul(
                        l2t[:, k * 512:(k + 1) * 512], sW2,
                        a1t[:, s * 2048 + k * 512: s * 2048 + (k + 1) * 512],
                        start=True, stop=True)
                nc.scalar.activation(e2t[:, s * 2048:(s + 1) * 2048], l2t,
                                     AF.Exp, bias=ldB2, scale=1.0)
            a2t = wk.tile([H, CW], mmdt, tag="a2", bufs=2, name=f"{rp}a2_{gg}")
            nc.scalar.activation(a2t, e2t, AF.Ln, bias=1.0, scale=1.0)
            a2s[gg] = a2t

        def psi_mms(gg, rp=""):
            a2t = a2s.pop(gg)
            for s in range(2):
                pr = ps.tile([1, 2048], f32, tag="mm",
                             name=f"{rp}psirow_{gg}_{s}")
                for k in range(4):
                    nc.tensor.matmul(
                        pr[0:1, k * 512:(k + 1) * 512], sWO[:, 0:1],
                        a2t[:, s * 2048 + k * 512: s * 2048 + (k + 1) * 512],
                        start=True, stop=True)
                pcols[(gg, s)] = pr

        def drains(gg, rp=""):
            for s in range(2):
                iA = gg * AB + 2 * s
                pr = pcols.pop((gg, s))
                row = wk.tile([1, 2048], f32, tag="rows", bufs=2,
                              name=f"{rp}row_{gg}_{s}")
                nc.vector.tensor_copy(row, pr)
                nc.sync.dma_start(psi[iA:iA + 1, :], row[0:1, 0:1024])
                nc.sync.dma_start(psi[iA + 1:iA + 2, :], row[0:1, 1024:2048])

        for rep in range(repeat):
            rp = f"r{rep}_"
            pre0_t = {}
            pre0_t[0] = wk.tile([H, CW], f32, tag="pre0", bufs=2,
                                name=rp + "pre0_0")
            preadd(0, pre0_t[0])
            for g in range(NGL):
                p0 = pre0_t.pop(g)
                nc.scalar.activation(p0, p0, AF.Exp)  # in place
                a0t = wk.tile([H, CW], mmdt, tag="a0", bufs=2,
                              name=f"{rp}a0_{g}")
                nc.scalar.activation(a0t, p0, AF.Ln, bias=1.0, scale=1.0)
                if g >= 3:
                    drains(g - 3, rp)
                if g + 1 < NGL:
                    pre0_t[g + 1] = wk.tile([H, CW], f32, tag="pre0", bufs=2,
                                            name=f"{rp}pre0_{g + 1}")
                    preadd(g + 1, pre0_t[g + 1])
                if g >= 1:
                    mlp_l12(g - 1, a0_prev, rp)
                if g >= 2:
                    psi_mms(g - 2, rp)
                a0_prev = a0t

            mlp_l12(NGL - 1, a0_prev, rp)

        # ---- tail: stabilized logmeanexp over j; split by partition range
        # so rows finished mid-loop overlap with the last groups ----
        # rows complete before the epilogue drains, 32-aligned (engine
        # partition offsets must be multiples of 32)
        PA = max(32, ((NGL - 4) * AB) // 32 * 32)
        slack = sb.tile([H, N], f32, name="slack")
        rowmax = sb.tile([H, 1], f32, name="rowmax")
        negb = sb.tile([H, 1], f32, name="negb")
        et = sb.tile([H, N], f32, name="et")
        rowsum = sb.tile([H, 1], f32, name="rowsum")
        lns = sb.tile([H, 1], f32, name="lns")
        v1 = sb.tile([H, 1], f32, name="v1")
        v2 = sb.tile([H, 1], f32, name="v2")
        psie = sb.tile([H, 1], f32, name="psie")

        def tail_part(lo, hi):
            nc.vector.tensor_sub(slack[lo:hi, :], cost[lo:hi, :],
                                 psi[lo:hi, :])
            nc.vector.reduce_max(rowmax[lo:hi], slack[lo:hi, :], axis=X_)
            nc.vector.tensor_scalar_mul(negb[lo:hi], rowmax[lo:hi], -10.0)
            nc.scalar.activation(et[lo:hi, :], slack[lo:hi, :], AF.Exp,
                                 bias=negb[lo:hi], scale=10.0,
                                 accum_out=rowsum[lo:hi])
            nc.scalar.activation(lns[lo:hi], rowsum[lo:hi], AF.Ln)
            nc.vector.tensor_scalar_mul(v1[lo:hi], lns[lo:hi], EPS)
            nc.vector.tensor_scalar_add(v2[lo:hi], v1[lo:hi],
                                        -EPS * math.log(float(N)))
            nc.vector.tensor_add(psie[lo:hi], v2[lo:hi], rowmax[lo:hi])

        rp = f"r{repeat - 1}_"
        tail_part(0, PA)
        psi_mms(NGL - 2, rp)
        psi_mms(NGL - 1, rp)
        drains(NGL - 3, rp)
        drains(NGL - 2, rp)
        drains(NGL - 1, rp)
        tail_part(PA, H)

        nc.sync.dma_start(dOUT[:], psie)

    _split_waits(nc, mybir)
    return nc


def _get_nc():
    import os
    variant = os.environ.get("KERNEL_VARIANT", "f32r")
    repeat = int(os.environ.get("KERNEL_REPEAT", "1"))
    ng = os.environ.get("KERNEL_NG")
    ng = int(ng) if ng else None
    key = ("nc", variant, repeat, ng)
    if key not in _cached:
        _cached[key] = _build(variant, repeat, ng)
    return _cached[key]


def kernel(**inputs):
    from concourse.bass_utils import run_bass_kernel_spmd

    X = np.asarray(inputs["X"], np.float32)
    U = np.asarray(inputs["U"], np.float32)
    Y = np.asarray(inputs["Y"], np.float32)
    W0 = np.asarray(inputs["W0"], np.float32)
    b0 = np.asarray(inputs["b0"], np.float32)
    W1 = np.asarray(inputs["W1"], np.float32)
    b1 = np.asarray(inputs["b1"], np.float32)
    W2 = np.asarray(inputs["W2"], np.float32)
    b2 = np.asarray(inputs["b2"], np.float32)
    Wout = np.asarray(inputs["Wout"], np.float32)
    bout = np.asarray(inputs["bout"], np.float32)

    YT = np.ascontiguousarray(Y.T)
    shared = {
        "YT": YT,
        "W0F": np.ascontiguousarray(W0[:F]),
        "W0R": np.ascontiguousarray(W0[F:]),
        "W1": np.ascontiguousarray(W1),
        "W2": np.ascontiguousarray(W2),
        "WOUT": np.ascontiguousarray(Wout),
        "B0": np.ascontiguousarray(b0[:, None]),
        "B1": np.ascontiguousarray(b1[:, None]),
        "B2": np.ascontiguousarray(b2[:, None]),
    }
    in_maps = []
    for c in range(NCORES):
        sl = slice(c * NLOC, (c + 1) * NLOC)
        m = dict(shared)
        m["XT"] = np.ascontiguousarray(X[sl].T)
        m["UT"] = np.ascontiguousarray(U[sl].T)
        in_maps.append(m)

    nc = _get_nc()
    res = run_bass_kernel_spmd(nc, in_maps, core_ids=list(range(NCORES)))
    _cached["last_res"] = res
    out = np.concatenate([res.results[c]["OUT"] for c in range(NCORES)], axis=0)
    return (out - bout[0]).astype(np.float32)


# revision 16
# speedup vs baseline: 1.3433x; 1.0019x over previous
"""Trainium2 Bass kernel for nn_EntropicOTQuantileRegression.

Math (reference):
    hX = X @ W0[:64]; hY = Y @ W0[64:]
    h  = sp(hX[i] + hY[j] + b0); h = sp(h@W1+b1); h = sp(h@W2+b2)
    psi[i,j] = (h @ Wout + bout); cost = U @ Y.T
    out[i] = eps*logmeanexp((cost[i,:]-psi[i,:])/eps)        (eps=0.1)

Sharding: 8 cores, data parallel over rows of X (128 rows each); Y/U-rows/
weights replicated or sharded accordingly; no collectives.

On-core layout: hidden dim (128) on partitions, pair columns on the free
axis. Per group of 4 local i's (4096 pair-columns):
    pre0 = hYT_b0 + hXT[:,i]          (DVE, per-i broadcast add)
    a0   = Ln(Exp(pre0)+1)            (ACT; softplus = exp+ln, no native op)
    l1   = W1.T @ a0 -> PSUM          (PE, fp32)
    a1   = Ln(Exp(l1 + b1)+1)         (ACT)
    l2, a2 likewise
    psi  = a2-chunk.T @ Wout          (PE transpose-trick: pairs on psum
                                       partitions, 128 pairs per matmul)
Tail: transpose psi back to [i, j] layout via PE transpose, then
rowmax/exp/sum/ln for the stabilized logmeanexp. bout folded in on host.

Constraint honored throughout: this toolchain's walrus accepts at most ONE
semaphore wait per compute instruction, so the pipeline is a strict
PE<->ACT ping-pong (DVE only feeds ACT / reads PE via already-waited
ticks), and the framework's 16-wait kernel-tail drain is monkeypatched
into a chain of single-wait drains.
"""
import math

import numpy as np

N = 1024
F = 64
R = 8
H = 128
NCORES = 8
NLOC = N // NCORES          # 128 local i rows per core
AB = 4                      # i's per act0 group
NG = NLOC // AB             # 32 groups (default; override via KERNEL_NG)
EPS = 0.1

_cached = {}


def _patch_drain():
    import concourse.tile as tile
    import concourse.mybir as mybir
    from concourse.vector_clock import ScopedClock

    if getattr(tile.TileContext, "_ant_drain_patched", False):
        return

    def _drain_and_barrier(self, tick_clock, wait_clock):
        nc = self.nc
        d0 = nc.sync.drain()
        wait_clock.add_sem_waits(
            d0.ins, ScopedClock({None: tick_clock.global_clock})
        )
        si = d0.ins.sync_info
        if si is not None and si.on_wait and len(si.on_wait) > 1:
            rest = list(si.on_wait[1:])
            d0.ins.sync_info = mybir.SyncInfo(
                on_wait=[si.on_wait[0]], on_update=list(si.on_update or [])
            )
            # spread the remaining waits across engines so they wait in
            # parallel; the all-engine barrier below joins them.
            engs = [nc.vector, nc.scalar, nc.tensor, nc.gpsimd, nc.sync]
            for idx, w in enumerate(rest):
                e = engs[idx % len(engs)]
                d = e.drain()
                d.ins.sync_info = mybir.SyncInfo(on_wait=[w], on_update=[])
        nc.all_engine_barrier()
        assert self.sems is not None
        popped = nc._tile_sem_poison_stack.pop()
        assert popped is self._sem_poison
        nc.clear_and_free_semaphores(list(self.sems.allocated().values()))
        nc.all_engine_barrier()

    tile.TileContext._drain_and_barrier = _drain_and_barrier
    tile.TileContext._ant_drain_patched = True


def _split_waits(nc, mybir):
    """Walrus in this toolchain accepts at most one semaphore wait per
    instruction; hoist extra waits onto injected same-engine NoOps."""
    n = 0
    for f in nc.m.functions:
        for bb in f.blocks:
            out = []
            for inst in bb.instructions:
                si = getattr(inst, "sync_info", None)
                if si is not None and si.on_wait and len(si.on_wait) > 1:
                    waits = list(si.on_wait)
                    for w in waits[:-1]:
                        out.append(mybir.InstNoOp(
                            name=f"antw-{nc.next_id()}",
                            engine=inst.engine,
                            sync_info=mybir.SyncInfo(on_wait=[w], on_update=[]),
                        ))
                    inst.sync_info = mybir.SyncInfo(
                        on_wait=[waits[-1]],
                        on_update=list(si.on_update or []),
                    )
                    n += 1
                out.append(inst)
            bb.instructions = out
    return n


def _build(variant="f32", repeat=1, ng=None):
    import concourse.bass as bass
    import concourse.tile as tile
    from concourse import mybir

    _patch_drain()
    NGL = NG if ng is None else ng

    f32 = mybir.dt.float32
    f32r = mybir.dt.float32r
    mmdt = f32r if variant == "f32r" else f32
    AF = mybir.ActivationFunctionType
    X_ = mybir.AxisListType.X

    nc = bass.Bass()
    dXT = nc.dram_tensor("XT", [F, NLOC], f32, kind="ExternalInput")
    dYT = nc.dram_tensor("YT", [R, N], f32, kind="ExternalInput")
    dUT = nc.dram_tensor("UT", [R, NLOC], f32, kind="ExternalInput")
    dW0F = nc.dram_tensor("W0F", [F, H], f32, kind="ExternalInput")
    dW0R = nc.dram_tensor("W0R", [R, H], f32, kind="ExternalInput")
    dW1 = nc.dram_tensor("W1", [H, H], f32, kind="ExternalInput")
    dW2 = nc.dram_tensor("W2", [H, H], f32, kind="ExternalInput")
    dWO = nc.dram_tensor("WOUT", [H, 1], f32, kind="ExternalInput")
    dB0 = nc.dram_tensor("B0", [H, 1], f32, kind="ExternalInput")
    dB1 = nc.dram_tensor("B1", [H, 1], f32, kind="ExternalInput")
    dB2 = nc.dram_tensor("B2", [H, 1], f32, kind="ExternalInput")
    dOUT = nc.dram_tensor("OUT", [NLOC, 1], f32, kind="ExternalOutput")

    with tile.TileContext(nc) as tc, \
         tc.tile_pool(name="sb", bufs=1) as sb, \
         tc.tile_pool(name="wk", bufs=1) as wk, \
         tc.tile_pool(name="ps", bufs=2, space=bass.MemorySpace.PSUM) as ps:

        # ---- loads (SWDGE) ----
        ldXT = sb.tile([F, NLOC], f32, name="ldXT")
        nc.gpsimd.dma_start(ldXT, dXT[:])
        ldYT = sb.tile([R, N], f32, name="ldYT")
        nc.gpsimd.dma_start(ldYT, dYT[:])
        ldUT = sb.tile([R, NLOC], f32, name="ldUT")
        nc.gpsimd.dma_start(ldUT, dUT[:])
        ldW0F = sb.tile([F, H], f32, name="ldW0F")
        nc.gpsimd.dma_start(ldW0F, dW0F[:])
        ldW0R = sb.tile([R, H], f32, name="ldW0R")
        nc.gpsimd.dma_start(ldW0R, dW0R[:])
        ldW1 = sb.tile([H, H], f32, name="ldW1")
        nc.gpsimd.dma_start(ldW1, dW1[:])
        ldW2 = sb.tile([H, H], f32, name="ldW2")
        nc.gpsimd.dma_start(ldW2, dW2[:])
        ldWO = sb.tile([H, 1], f32, name="ldWO")
        nc.gpsimd.dma_start(ldWO, dWO[:])
        ldB0 = sb.tile([H, 1], f32, name="ldB0")
        nc.gpsimd.dma_start(ldB0, dB0[:])
        ldB1 = sb.tile([H, 1], f32, name="ldB1")
        nc.gpsimd.dma_start(ldB1, dB1[:])
        ldB2 = sb.tile([H, 1], f32, name="ldB2")
        nc.gpsimd.dma_start(ldB2, dB2[:])

        # f32r roundings for the big matmuls (DVE converts on copy)
        sW1 = sb.tile([H, H], mmdt, name="sW1")
        nc.vector.tensor_copy(sW1, ldW1)
        sW2 = sb.tile([H, H], mmdt, name="sW2")
        nc.vector.tensor_copy(sW2, ldW2)
        sWO = sb.tile([H, 1], mmdt, name="sWO")
        nc.vector.tensor_copy(sWO, ldWO)

        # ---- preamble (plain f32 matmuls; small) ----
        hXTp = ps.tile([H, NLOC], f32, tag="mm", name="hXTp")
        nc.tensor.matmul(hXTp, ldW0F, ldXT, start=True, stop=True)
        hXT = sb.tile([H, NLOC], f32, name="hXT")
        nc.vector.tensor_copy(hXT, hXTp)

        hYp = ps.tile([H, N], f32, tag="mm", name="hYp")
        for k in range(2):
            nc.tensor.matmul(hYp[:, k * 512:(k + 1) * 512], ldW0R,
                             ldYT[:, k * 512:(k + 1) * 512],
                             start=True, stop=True)
        hYb0 = sb.tile([H, N], f32, name="hYb0")
        nc.scalar.activation(hYb0, hYp, AF.Identity, bias=ldB0, scale=1.0)

        costp = ps.tile([H, N], f32, tag="mm", name="costp")
        for k in range(2):
            nc.tensor.matmul(costp[:, k * 512:(k + 1) * 512], ldUT,
                             ldYT[:, k * 512:(k + 1) * 512],
                             start=True, stop=True)
        cost = sb.tile([H, N], f32, name="cost")
        nc.scalar.copy(cost, costp)

        psi = sb.tile([H, N], f32, name="psi")

        CW = AB * N  # columns per group

        def preadd(g, tgt):
            for k in range(AB):
                i = g * AB + k
                nc.vector.tensor_scalar_add(
                    tgt[:, k * N:(k + 1) * N], hYb0, hXT[:, i:i + 1])

        pcols = {}
        a2s = {}

        def mlp_l12(gg, a0t, rp=""):
            e1t = wk.tile([H, CW], f32, tag="e1", bufs=1, name=f"{rp}e1_{gg}")
            for s in range(2):
                l1t = ps.tile([H, 2048], f32, tag="mm", name=f"{rp}l1_{gg}_{s}")
                for k in range(4):
                    nc.tensor.matmul(
                        l1t[:, k * 512:(k + 1) * 512], sW1,
                        a0t[:, s * 2048 + k * 512: s * 2048 + (k + 1) * 512],
                        start=True, stop=True)
                nc.scalar.activation(e1t[:, s * 2048:(s + 1) * 2048], l1t,
                                     AF.Exp, bias=ldB1, scale=1.0)
            a1t = wk.tile([H, CW], mmdt, tag="a1", bufs=1, name=f"{rp}a1_{gg}")
            if gg >= NGL - 5:
                # late groups: halve ln1 so l2(s0) unblocks sooner
                for s in range(2):
                    nc.scalar.activation(a1t[:, s * 2048:(s + 1) * 2048],
                                         e1t[:, s * 2048:(s + 1) * 2048],
                                         AF.Ln, bias=1.0, scale=1.0)
            else:
                nc.scalar.activation(a1t, e1t, AF.Ln, bias=1.0, scale=1.0)

            e2t = wk.tile([H, CW], f32, tag="e2", bufs=1, name=f"{rp}e2_{gg}")
            for s in range(2):
                l2t = ps.tile([H, 2048], f32, tag="mm", name=f"{rp}l2_{gg}_{s}")
                for k in range(4):
                    nc.tensor.matmul(
                        l2t[:, k * 512:(k + 1) * 512], sW2,
                        a1t[:, s * 2048 + k * 512: s * 2048 + (k + 1) * 512],
                        start=True, stop=True)
                nc.scalar.activation(e2t[:, s * 2048:(s + 1) * 2048], l2t,
                                     AF.Exp, bias=ldB2, scale=1.0)
            a2t = wk.tile([H, CW], mmdt, tag="a2", bufs=2, name=f"{rp}a2_{gg}")
            nc.scalar.activation(a2t, e2t, AF.Ln, bias=1.0, scale=1.0)
            a2s[gg] = a2t

        def psi_mms(gg, rp=""):
            a2t = a2s.pop(gg)
            for s in range(2):
                pr = ps.tile([1, 2048], f32, tag="mm",
                             name=f"{rp}psirow_{gg}_{s}")
                for k in range(4):
                    nc.tensor.matmul(
                        pr[0:1, k * 512:(k + 1) * 512], sWO[:, 0:1],
                        a2t[:, s * 2048 + k * 512: s * 2048 + (k + 1) * 512],
                        start=True, stop=True)
                pcols[(gg, s)] = pr

        def drains(gg, rp=""):
            for s in range(2):
                iA = gg * AB + 2 * s
                pr = pcols.pop((gg, s))
                row = wk.tile([1, 2048], f32, tag="rows", bufs=2,
                              name=f"{rp}row_{gg}_{s}")
                nc.vector.tensor_copy(row, pr)
                nc.sync.dma_start(psi[iA:iA + 1, :], row[0:1, 0:1024])
                nc.sync.dma_start(psi[iA + 1:iA + 2, :], row[0:1, 1024:2048])

        for rep in range(repeat):
            rp = f"r{rep}_"
            pre0_t = {}
            pre0_t[0] = wk.tile([H, CW], f32, tag="pre0", bufs=2,
                                name=rp + "pre0_0")
            preadd(0, pre0_t[0])
            for g in range(NGL):
                p0 = pre0_t.pop(g)
                nc.scalar.activation(p0, p0, AF.Exp)  # in place
                a0t = wk.tile([H, CW], mmdt, tag="a0", bufs=2,
                              name=f"{rp}a0_{g}")
                nc.scalar.activation(a0t, p0, AF.Ln, bias=1.0, scale=1.0)
                if g >= 3:
                    drains(g - 3, rp)
                if g + 1 < NGL:
                    pre0_t[g + 1] = wk.tile([H, CW], f32, tag="pre0", bufs=2,
                                            name=f"{rp}pre0_{g + 1}")
                    preadd(g + 1, pre0_t[g + 1])
                if g >= 1:
                    mlp_l12(g - 1, a0_prev, rp)
                if g >= 2:
                    psi_mms(g - 2, rp)
                a0_prev = a0t

            mlp_l12(NGL - 1, a0_prev, rp)

        # ---- tail: stabilized logmeanexp over j; split by partition range
        # so rows finished mid-loop overlap with the last groups ----
        # rows complete before the epilogue drains, 32-aligned (engine
        # partition offsets must be multiples of 32)
        PA = max(32, ((NGL - 4) * AB) // 32 * 32)
        slack = sb.tile([H, N], f32, name="slack")
        rowmax = sb.tile([H, 1], f32, name="rowmax")
        negb = sb.tile([H, 1], f32, name="negb")
        et = sb.tile([H, N], f32, name="et")
        rowsum = sb.tile([H, 1], f32, name="rowsum")
        lns = sb.tile([H, 1], f32, name="lns")
        v1 = sb.tile([H, 1], f32, name="v1")
        v2 = sb.tile([H, 1], f32, name="v2")
        psie = sb.tile([H, 1], f32, name="psie")

        def tail_part(lo, hi):
            nc.vector.tensor_sub(slack[lo:hi, :], cost[lo:hi, :],
                                 psi[lo:hi, :])
            nc.vector.reduce_max(rowmax[lo:hi], slack[lo:hi, :], axis=X_)
            nc.vector.tensor_scalar_mul(negb[lo:hi], rowmax[lo:hi], -10.0)
            nc.scalar.activation(et[lo:hi, :], slack[lo:hi, :], AF.Exp,
                                 bias=negb[lo:hi], scale=10.0,
                                 accum_out=rowsum[lo:hi])
            nc.scalar.activation(lns[lo:hi], rowsum[lo:hi], AF.Ln)
            nc.vector.tensor_scalar_mul(v1[lo:hi], lns[lo:hi], EPS)
            nc.vector.tensor_scalar_add(v2[lo:hi], v1[lo:hi],
                                        -EPS * math.log(float(N)))
            nc.vector.tensor_add(psie[lo:hi], v2[lo:hi], rowmax[lo:hi])

        rp = f"r{repeat - 1}_"
        psi_mms(NGL - 2, rp)
        drains(NGL - 3, rp)
        drains(NGL - 2, rp)
        tail_part(0, PA)
        psi_mms(NGL - 1, rp)
        drains(NGL - 1, rp)
        tail_part(PA, H)

        nc.sync.dma_start(dOUT[:], psie)

    _split_waits(nc, mybir)
    return nc


def _get_nc():
    import os
    variant = os.environ.get("KERNEL_VARIANT", "f32r")
    repeat = int(os.environ.get("KERNEL_REPEAT", "1"))
    ng = os.environ.get("KERNEL_NG")
    ng = int(ng) if ng else None
    key = ("nc", variant, repeat, ng)
    if key not in _cached:
        _cached[key] = _build(variant, repeat, ng)
    return _cached[key]


def kernel(**inputs):
    from concourse.bass_utils import run_bass_kernel_spmd

    X = np.asarray(inputs["X"], np.float32)
    U = np.asarray(inputs["U"], np.float32)
    Y = np.asarray(inputs["Y"], np.float32)
    W0 = np.asarray(inputs["W0"], np.float32)
    b0 = np.asarray(inputs["b0"], np.float32)
    W1 = np.asarray(inputs["W1"], np.float32)
    b1 = np.asarray(inputs["b1"], np.float32)
    W2 = np.asarray(inputs["W2"], np.float32)
    b2 = np.asarray(inputs["b2"], np.float32)
    Wout = np.asarray(inputs["Wout"], np.float32)
    bout = np.asarray(inputs["bout"], np.float32)

    YT = np.ascontiguousarray(Y.T)
    shared = {
        "YT": YT,
        "W0F": np.ascontiguousarray(W0[:F]),
        "W0R": np.ascontiguousarray(W0[F:]),
        "W1": np.ascontiguousarray(W1),
        "W2": np.ascontiguousarray(W2),
        "WOUT": np.ascontiguousarray(Wout),
        "B0": np.ascontiguousarray(b0[:, None]),
        "B1": np.ascontiguousarray(b1[:, None]),
        "B2": np.ascontiguousarray(b2[:, None]),
    }
    in_maps = []
    for c in range(NCORES):
        sl = slice(c * NLOC, (c + 1) * NLOC)
        m = dict(shared)
        m["XT"] = np.ascontiguousarray(X[sl].T)
        m["UT"] = np.ascontiguousarray(U[sl].T)
        in_maps.append(m)

    nc = _get_nc()
    res = run_bass_kernel_spmd(nc, in_maps, core_ids=list(range(NCORES)))
    _cached["last_res"] = res
    out = np.concatenate([res.results[c]["OUT"] for c in range(NCORES)], axis=0)
    return (out - bout[0]).astype(np.float32)


# revision 19
# speedup vs baseline: 1.3806x; 1.0277x over previous
"""Trainium2 Bass kernel for nn_EntropicOTQuantileRegression.

Math (reference):
    hX = X @ W0[:64]; hY = Y @ W0[64:]
    h  = sp(hX[i] + hY[j] + b0); h = sp(h@W1+b1); h = sp(h@W2+b2)
    psi[i,j] = (h @ Wout + bout); cost = U @ Y.T
    out[i] = eps*logmeanexp((cost[i,:]-psi[i,:])/eps)        (eps=0.1)

Sharding: 8 cores, data parallel over rows of X (128 rows each); Y/U-rows/
weights replicated or sharded accordingly; no collectives.

On-core layout: hidden dim (128) on partitions, pair columns on the free
axis. Per group of 4 local i's (4096 pair-columns):
    pre0 = hYT_b0 + hXT[:,i]          (DVE, per-i broadcast add)
    a0   = Ln(Exp(pre0)+1)            (ACT; softplus = exp+ln, no native op)
    l1   = W1.T @ a0 -> PSUM          (PE, fp32)
    a1   = Ln(Exp(l1 + b1)+1)         (ACT)
    l2, a2 likewise
    psi  = a2-chunk.T @ Wout          (PE transpose-trick: pairs on psum
                                       partitions, 128 pairs per matmul)
Tail: transpose psi back to [i, j] layout via PE transpose, then
rowmax/exp/sum/ln for the stabilized logmeanexp. bout folded in on host.

Constraint honored throughout: this toolchain's walrus accepts at most ONE
semaphore wait per compute instruction, so the pipeline is a strict
PE<->ACT ping-pong (DVE only feeds ACT / reads PE via already-waited
ticks), and the framework's 16-wait kernel-tail drain is monkeypatched
into a chain of single-wait drains.
"""
import math

import numpy as np

N = 1024
F = 64
R = 8
H = 128
NCORES = 8
NLOC = N // NCORES          # 128 local i rows per core
AB = 4                      # i's per act0 group
NG = NLOC // AB             # 32 groups (default; override via KERNEL_NG)
EPS = 0.1

_cached = {}


def _patch_drain():
    import concourse.tile as tile
    import concourse.mybir as mybir
    from concourse.vector_clock import ScopedClock

    if getattr(tile.TileContext, "_ant_drain_patched", False):
        return

    def _drain_and_barrier(self, tick_clock, wait_clock):
        nc = self.nc
        d0 = nc.sync.drain()
        wait_clock.add_sem_waits(
            d0.ins, ScopedClock({None: tick_clock.global_clock})
        )
        si = d0.ins.sync_info
        if si is not None and si.on_wait and len(si.on_wait) > 1:
            rest = list(si.on_wait[1:])
            d0.ins.sync_info = mybir.SyncInfo(
                on_wait=[si.on_wait[0]], on_update=list(si.on_update or [])
            )
            # spread the remaining waits across engines so they wait in
            # parallel; the all-engine barrier below joins them.
            engs = [nc.vector, nc.scalar, nc.tensor, nc.gpsimd, nc.sync]
            for idx, w in enumerate(rest):
                e = engs[idx % len(engs)]
                d = e.drain()
                d.ins.sync_info = mybir.SyncInfo(on_wait=[w], on_update=[])
        nc.all_engine_barrier()
        assert self.sems is not None
        popped = nc._tile_sem_poison_stack.pop()
        assert popped is self._sem_poison
        nc.clear_and_free_semaphores(list(self.sems.allocated().values()))
        nc.all_engine_barrier()

    tile.TileContext._drain_and_barrier = _drain_and_barrier
    tile.TileContext._ant_drain_patched = True


def _split_waits(nc, mybir):
    """Walrus in this toolchain accepts at most one semaphore wait per
    instruction; hoist extra waits onto injected same-engine NoOps."""
    n = 0
    for f in nc.m.functions:
        for bb in f.blocks:
            out = []
            for inst in bb.instructions:
                si = getattr(inst, "sync_info", None)
                if si is not None and si.on_wait and len(si.on_wait) > 1:
                    waits = list(si.on_wait)
                    for w in waits[:-1]:
                        out.append(mybir.InstNoOp(
                            name=f"antw-{nc.next_id()}",
                            engine=inst.engine,
                            sync_info=mybir.SyncInfo(on_wait=[w], on_update=[]),
                        ))
                    inst.sync_info = mybir.SyncInfo(
                        on_wait=[waits[-1]],
                        on_update=list(si.on_update or []),
                    )
                    n += 1
                out.append(inst)
            bb.instructions = out
    return n


def _build(variant="f32", repeat=1, ng=None):
    import concourse.bass as bass
    import concourse.tile as tile
    from concourse import mybir

    _patch_drain()
    NGL = NG if ng is None else ng

    f32 = mybir.dt.float32
    f32r = mybir.dt.float32r
    mmdt = f32r if variant == "f32r" else f32
    AF = mybir.ActivationFunctionType
    X_ = mybir.AxisListType.X

    nc = bass.Bass()
    dXT = nc.dram_tensor("XT", [F, NLOC], f32, kind="ExternalInput")
    dYT = nc.dram_tensor("YT", [R, N], f32, kind="ExternalInput")
    dUT = nc.dram_tensor("UT", [R, NLOC], f32, kind="ExternalInput")
    dW0F = nc.dram_tensor("W0F", [F, H], f32, kind="ExternalInput")
    dW0R = nc.dram_tensor("W0R", [R, H], f32, kind="ExternalInput")
    dW1 = nc.dram_tensor("W1", [H, H], f32, kind="ExternalInput")
    dW2 = nc.dram_tensor("W2", [H, H], f32, kind="ExternalInput")
    dWO = nc.dram_tensor("WOUT", [H, 1], f32, kind="ExternalInput")
    dB0 = nc.dram_tensor("B0", [H, 1], f32, kind="ExternalInput")
    dB1 = nc.dram_tensor("B1", [H, 1], f32, kind="ExternalInput")
    dB2 = nc.dram_tensor("B2", [H, 1], f32, kind="ExternalInput")
    dOUT = nc.dram_tensor("OUT", [NLOC, 1], f32, kind="ExternalOutput")

    with tile.TileContext(nc) as tc, \
         tc.tile_pool(name="sb", bufs=1) as sb, \
         tc.tile_pool(name="wk", bufs=1) as wk, \
         tc.tile_pool(name="ps", bufs=2, space=bass.MemorySpace.PSUM) as ps:

        # ---- loads (SWDGE) ----
        ldXT = sb.tile([F, NLOC], f32, name="ldXT")
        nc.gpsimd.dma_start(ldXT, dXT[:])
        ldYT = sb.tile([R, N], f32, name="ldYT")
        nc.gpsimd.dma_start(ldYT, dYT[:])
        ldUT = sb.tile([R, NLOC], f32, name="ldUT")
        nc.gpsimd.dma_start(ldUT, dUT[:])
        ldW0F = sb.tile([F, H], f32, name="ldW0F")
        nc.gpsimd.dma_start(ldW0F, dW0F[:])
        ldW0R = sb.tile([R, H], f32, name="ldW0R")
        nc.gpsimd.dma_start(ldW0R, dW0R[:])
        ldW1 = sb.tile([H, H], f32, name="ldW1")
        nc.gpsimd.dma_start(ldW1, dW1[:])
        ldW2 = sb.tile([H, H], f32, name="ldW2")
        nc.gpsimd.dma_start(ldW2, dW2[:])
        ldWO = sb.tile([H, 1], f32, name="ldWO")
        nc.gpsimd.dma_start(ldWO, dWO[:])
        ldB0 = sb.tile([H, 1], f32, name="ldB0")
        nc.gpsimd.dma_start(ldB0, dB0[:])
        ldB1 = sb.tile([H, 1], f32, name="ldB1")
        nc.gpsimd.dma_start(ldB1, dB1[:])
        ldB2 = sb.tile([H, 1], f32, name="ldB2")
        nc.gpsimd.dma_start(ldB2, dB2[:])

        # f32r roundings for the big matmuls (DVE converts on copy)
        sW1 = sb.tile([H, H], mmdt, name="sW1")
        nc.vector.tensor_copy(sW1, ldW1)
        sW2 = sb.tile([H, H], mmdt, name="sW2")
        nc.vector.tensor_copy(sW2, ldW2)
        sWO = sb.tile([H, 1], mmdt, name="sWO")
        nc.vector.tensor_copy(sWO, ldWO)

        # ---- preamble (plain f32 matmuls; small) ----
        hXTp = ps.tile([H, NLOC], f32, tag="mm", name="hXTp")
        nc.tensor.matmul(hXTp, ldW0F, ldXT, start=True, stop=True)
        hXT = sb.tile([H, NLOC], f32, name="hXT")
        nc.vector.tensor_copy(hXT, hXTp)

        hYp = ps.tile([H, N], f32, tag="mm", name="hYp")
        for k in range(2):
            nc.tensor.matmul(hYp[:, k * 512:(k + 1) * 512], ldW0R,
                             ldYT[:, k * 512:(k + 1) * 512],
                             start=True, stop=True)
        hYb0 = sb.tile([H, N], f32, name="hYb0")
        nc.scalar.activation(hYb0, hYp, AF.Identity, bias=ldB0, scale=1.0)

        costp = ps.tile([H, N], f32, tag="mm", name="costp")
        for k in range(2):
            nc.tensor.matmul(costp[:, k * 512:(k + 1) * 512], ldUT,
                             ldYT[:, k * 512:(k + 1) * 512],
                             start=True, stop=True)
        cost = sb.tile([H, N], f32, name="cost")
        nc.scalar.copy(cost, costp)

        psi = sb.tile([H, N], f32, name="psi")

        CW = AB * N  # columns per group

        def preadd(g, tgt):
            for k in range(AB):
                i = g * AB + k
                nc.vector.tensor_scalar_add(
                    tgt[:, k * N:(k + 1) * N], hYb0, hXT[:, i:i + 1])

        pcols = {}
        a2s = {}

        def mlp_l12(gg, a0t, rp=""):
            e1t = wk.tile([H, CW], f32, tag="e1", bufs=1, name=f"{rp}e1_{gg}")
            for s in range(2):
                l1t = ps.tile([H, 2048], f32, tag="mm", name=f"{rp}l1_{gg}_{s}")
                for k in range(4):
                    nc.tensor.matmul(
                        l1t[:, k * 512:(k + 1) * 512], sW1,
                        a0t[:, s * 2048 + k * 512: s * 2048 + (k + 1) * 512],
                        start=True, stop=True)
                nc.scalar.activation(e1t[:, s * 2048:(s + 1) * 2048], l1t,
                                     AF.Exp, bias=ldB1, scale=1.0)
            a1t = wk.tile([H, CW], mmdt, tag="a1", bufs=1, name=f"{rp}a1_{gg}")
            if True:
                # late groups: halve ln1 so l2(s0) unblocks sooner
                for s in range(2):
                    nc.scalar.activation(a1t[:, s * 2048:(s + 1) * 2048],
                                         e1t[:, s * 2048:(s + 1) * 2048],
                                         AF.Ln, bias=1.0, scale=1.0)
            else:
                nc.scalar.activation(a1t, e1t, AF.Ln, bias=1.0, scale=1.0)

            e2t = wk.tile([H, CW], f32, tag="e2", bufs=1, name=f"{rp}e2_{gg}")
            for s in range(2):
                l2t = ps.tile([H, 2048], f32, tag="mm", name=f"{rp}l2_{gg}_{s}")
                for k in range(4):
                    nc.tensor.matmul(
                        l2t[:, k * 512:(k + 1) * 512], sW2,
                        a1t[:, s * 2048 + k * 512: s * 2048 + (k + 1) * 512],
                        start=True, stop=True)
                nc.scalar.activation(e2t[:, s * 2048:(s + 1) * 2048], l2t,
                                     AF.Exp, bias=ldB2, scale=1.0)
            a2t = wk.tile([H, CW], mmdt, tag="a2", bufs=2, name=f"{rp}a2_{gg}")
            nc.scalar.activation(a2t, e2t, AF.Ln, bias=1.0, scale=1.0)
            a2s[gg] = a2t

        def psi_mms(gg, rp=""):
            a2t = a2s.pop(gg)
            for s in range(2):
                pr = ps.tile([1, 2048], f32, tag="mm",
                             name=f"{rp}psirow_{gg}_{s}")
                for k in range(4):
                    nc.tensor.matmul(
                        pr[0:1, k * 512:(k + 1) * 512], sWO[:, 0:1],
                        a2t[:, s * 2048 + k * 512: s * 2048 + (k + 1) * 512],
                        start=True, stop=True)
                pcols[(gg, s)] = pr

        def drains(gg, rp=""):
            for s in range(2):
                iA = gg * AB + 2 * s
                pr = pcols.pop((gg, s))
                row = wk.tile([1, 2048], f32, tag="rows", bufs=2,
                              name=f"{rp}row_{gg}_{s}")
                nc.vector.tensor_copy(row, pr)
                nc.sync.dma_start(psi[iA:iA + 1, :], row[0:1, 0:1024])
                nc.sync.dma_start(psi[iA + 1:iA + 2, :], row[0:1, 1024:2048])

        for rep in range(repeat):
            rp = f"r{rep}_"
            pre0_t = {}
            pre0_t[0] = wk.tile([H, CW], f32, tag="pre0", bufs=2,
                                name=rp + "pre0_0")
            preadd(0, pre0_t[0])
            for g in range(NGL):
                p0 = pre0_t.pop(g)
                nc.scalar.activation(p0, p0, AF.Exp)  # in place
                a0t = wk.tile([H, CW], mmdt, tag="a0", bufs=2,
                              name=f"{rp}a0_{g}")
                nc.scalar.activation(a0t, p0, AF.Ln, bias=1.0, scale=1.0)
                if g >= 3:
                    drains(g - 3, rp)
                if g + 1 < NGL:
                    pre0_t[g + 1] = wk.tile([H, CW], f32, tag="pre0", bufs=2,
                                            name=f"{rp}pre0_{g + 1}")
                    preadd(g + 1, pre0_t[g + 1])
                if g >= 1:
                    mlp_l12(g - 1, a0_prev, rp)
                if g >= 2:
                    psi_mms(g - 2, rp)
                a0_prev = a0t

            mlp_l12(NGL - 1, a0_prev, rp)

        # ---- tail: stabilized logmeanexp over j; split by partition range
        # so rows finished mid-loop overlap with the last groups ----
        # rows complete before the epilogue drains, 32-aligned (engine
        # partition offsets must be multiples of 32)
        PA = max(32, ((NGL - 4) * AB) // 32 * 32)
        slack = sb.tile([H, N], f32, name="slack")
        rowmax = sb.tile([H, 1], f32, name="rowmax")
        negb = sb.tile([H, 1], f32, name="negb")
        et = sb.tile([H, N], f32, name="et")
        rowsum = sb.tile([H, 1], f32, name="rowsum")
        lns = sb.tile([H, 1], f32, name="lns")
        v1 = sb.tile([H, 1], f32, name="v1")
        v2 = sb.tile([H, 1], f32, name="v2")
        psie = sb.tile([H, 1], f32, name="psie")

        def tail_part(lo, hi):
            nc.vector.tensor_sub(slack[lo:hi, :], cost[lo:hi, :],
                                 psi[lo:hi, :])
            nc.vector.reduce_max(rowmax[lo:hi], slack[lo:hi, :], axis=X_)
            nc.vector.tensor_scalar_mul(negb[lo:hi], rowmax[lo:hi], -10.0)
            nc.scalar.activation(et[lo:hi, :], slack[lo:hi, :], AF.Exp,
                                 bias=negb[lo:hi], scale=10.0,
                                 accum_out=rowsum[lo:hi])
            nc.scalar.activation(lns[lo:hi], rowsum[lo:hi], AF.Ln)
            nc.vector.tensor_scalar_mul(v1[lo:hi], lns[lo:hi], EPS)
            nc.vector.tensor_scalar_add(v2[lo:hi], v1[lo:hi],
                                        -EPS * math.log(float(N)))
            nc.vector.tensor_add(psie[lo:hi], v2[lo:hi], rowmax[lo:hi])

        rp = f"r{repeat - 1}_"
        psi_mms(NGL - 2, rp)
        drains(NGL - 3, rp)
        drains(NGL - 2, rp)
        tail_part(0, PA)
        psi_mms(NGL - 1, rp)
        drains(NGL - 1, rp)
        tail_part(PA, H)

        nc.sync.dma_start(dOUT[:], psie)

    _split_waits(nc, mybir)
    return nc


def _get_nc():
    import os
    variant = os.environ.get("KERNEL_VARIANT", "f32r")
    repeat = int(os.environ.get("KERNEL_REPEAT", "1"))
    ng = os.environ.get("KERNEL_NG")
    ng = int(ng) if ng else None
    key = ("nc", variant, repeat, ng)
    if key not in _cached:
        _cached[key] = _build(variant, repeat, ng)
    return _cached[key]


def kernel(**inputs):
    from concourse.bass_utils import run_bass_kernel_spmd

    X = np.asarray(inputs["X"], np.float32)
    U = np.asarray(inputs["U"], np.float32)
    Y = np.asarray(inputs["Y"], np.float32)
    W0 = np.asarray(inputs["W0"], np.float32)
    b0 = np.asarray(inputs["b0"], np.float32)
    W1 = np.asarray(inputs["W1"], np.float32)
    b1 = np.asarray(inputs["b1"], np.float32)
    W2 = np.asarray(inputs["W2"], np.float32)
    b2 = np.asarray(inputs["b2"], np.float32)
    Wout = np.asarray(inputs["Wout"], np.float32)
    bout = np.asarray(inputs["bout"], np.float32)

    YT = np.ascontiguousarray(Y.T)
    shared = {
        "YT": YT,
        "W0F": np.ascontiguousarray(W0[:F]),
        "W0R": np.ascontiguousarray(W0[F:]),
        "W1": np.ascontiguousarray(W1),
        "W2": np.ascontiguousarray(W2),
        "WOUT": np.ascontiguousarray(Wout),
        "B0": np.ascontiguousarray(b0[:, None]),
        "B1": np.ascontiguousarray(b1[:, None]),
        "B2": np.ascontiguousarray(b2[:, None]),
    }
    in_maps = []
    for c in range(NCORES):
        sl = slice(c * NLOC, (c + 1) * NLOC)
        m = dict(shared)
        m["XT"] = np.ascontiguousarray(X[sl].T)
        m["UT"] = np.ascontiguousarray(U[sl].T)
        in_maps.append(m)

    nc = _get_nc()
    res = run_bass_kernel_spmd(nc, in_maps, core_ids=list(range(NCORES)))
    _cached["last_res"] = res
    out = np.concatenate([res.results[c]["OUT"] for c in range(NCORES)], axis=0)
    return (out - bout[0]).astype(np.float32)


# revision 22
# speedup vs baseline: 1.3819x; 1.0009x over previous
"""Trainium2 Bass kernel for nn_EntropicOTQuantileRegression.

Math (reference):
    hX = X @ W0[:64]; hY = Y @ W0[64:]
    h  = sp(hX[i] + hY[j] + b0); h = sp(h@W1+b1); h = sp(h@W2+b2)
    psi[i,j] = (h @ Wout + bout); cost = U @ Y.T
    out[i] = eps*logmeanexp((cost[i,:]-psi[i,:])/eps)        (eps=0.1)

Sharding: 8 cores, data parallel over rows of X (128 rows each); Y/U-rows/
weights replicated or sharded accordingly; no collectives.

On-core layout: hidden dim (128) on partitions, pair columns on the free
axis. Per group of 4 local i's (4096 pair-columns):
    pre0 = hYT_b0 + hXT[:,i]          (DVE, per-i broadcast add)
    a0   = Ln(Exp(pre0)+1)            (ACT; softplus = exp+ln, no native op;
                                       Exp runs in place on pre0)
    l1   = W1.T @ a0 -> PSUM          (PE, f32r, 2x[128,2048] psum slots)
    a1   = Ln(Exp(l1 + b1)+1)         (ACT; ln halved per subgroup so l2
                                       unblocks sooner)
    l2, a2 likewise
    psi  = Wout.T @ a2 -> psum row    (PE, M=1 row matmuls, deferred one
                                       group; DVE drains the [1,2048] rows,
                                       SBUF->SBUF DMA scatters them to the
                                       per-i partitions of psi)
Tail: stabilized logmeanexp over j (rowmax via DVE reduce, Exp with
accum_out row sums, Ln), split at partition 96 so most rows overlap the
last groups. bout folded in on host. ACT (ScalarE) is the bottleneck
(~730 us busy of ~754 us total); the schedule keeps it ~95% utilized.

Toolchain constraints handled here: walrus accepts at most ONE semaphore
wait per instruction (_split_waits hoists extras onto injected NoOps; the
16-wait kernel-tail drain is split across engines), f32r producers must
write f32r-typed tiles, engine APs need 32-aligned partition offsets, and
DMA cannot touch PSUM.
"""
import math

import numpy as np

N = 1024
F = 64
R = 8
H = 128
NCORES = 8
NLOC = N // NCORES          # 128 local i rows per core
AB = 4                      # i's per act0 group
NG = NLOC // AB             # 32 groups (default; override via KERNEL_NG)
EPS = 0.1

_cached = {}


def _patch_drain():
    import concourse.tile as tile
    import concourse.mybir as mybir
    from concourse.vector_clock import ScopedClock

    if getattr(tile.TileContext, "_ant_drain_patched", False):
        return

    def _drain_and_barrier(self, tick_clock, wait_clock):
        nc = self.nc
        d0 = nc.sync.drain()
        wait_clock.add_sem_waits(
            d0.ins, ScopedClock({None: tick_clock.global_clock})
        )
        si = d0.ins.sync_info
        if si is not None and si.on_wait and len(si.on_wait) > 1:
            rest = list(si.on_wait[1:])
            d0.ins.sync_info = mybir.SyncInfo(
                on_wait=[si.on_wait[0]], on_update=list(si.on_update or [])
            )
            # spread the remaining waits across engines so they wait in
            # parallel; the all-engine barrier below joins them.
            engs = [nc.vector, nc.scalar, nc.tensor, nc.gpsimd, nc.sync]
            for idx, w in enumerate(rest):
                e = engs[idx % len(engs)]
                d = e.drain()
                d.ins.sync_info = mybir.SyncInfo(on_wait=[w], on_update=[])
        nc.all_engine_barrier()
        assert self.sems is not None
        popped = nc._tile_sem_poison_stack.pop()
        assert popped is self._sem_poison
        nc.clear_and_free_semaphores(list(self.sems.allocated().values()))
        nc.all_engine_barrier()

    tile.TileContext._drain_and_barrier = _drain_and_barrier
    tile.TileContext._ant_drain_patched = True


def _split_waits(nc, mybir):
    """Walrus in this toolchain accepts at most one semaphore wait per
    instruction; hoist extra waits onto injected same-engine NoOps."""
    n = 0
    for f in nc.m.functions:
        for bb in f.blocks:
            out = []
            for inst in bb.instructions:
                si = getattr(inst, "sync_info", None)
                if si is not None and si.on_wait and len(si.on_wait) > 1:
                    waits = list(si.on_wait)
                    for w in waits[:-1]:
                        out.append(mybir.InstNoOp(
                            name=f"antw-{nc.next_id()}",
                            engine=inst.engine,
                            sync_info=mybir.SyncInfo(on_wait=[w], on_update=[]),
                        ))
                    inst.sync_info = mybir.SyncInfo(
                        on_wait=[waits[-1]],
                        on_update=list(si.on_update or []),
                    )
                    n += 1
                out.append(inst)
            bb.instructions = out
    return n


def _build(variant="f32", repeat=1, ng=None):
    import concourse.bass as bass
    import concourse.tile as tile
    from concourse import mybir

    _patch_drain()
    NGL = NG if ng is None else ng

    f32 = mybir.dt.float32
    f32r = mybir.dt.float32r
    mmdt = f32r if variant == "f32r" else f32
    AF = mybir.ActivationFunctionType
    X_ = mybir.AxisListType.X

    nc = bass.Bass()
    dXT = nc.dram_tensor("XT", [F, NLOC], f32, kind="ExternalInput")
    dYT = nc.dram_tensor("YT", [R, N], f32, kind="ExternalInput")
    dUT = nc.dram_tensor("UT", [R, NLOC], f32, kind="ExternalInput")
    dW0F = nc.dram_tensor("W0F", [F, H], f32, kind="ExternalInput")
    dW0R = nc.dram_tensor("W0R", [R, H], f32, kind="ExternalInput")
    dW1 = nc.dram_tensor("W1", [H, H], f32, kind="ExternalInput")
    dW2 = nc.dram_tensor("W2", [H, H], f32, kind="ExternalInput")
    dWO = nc.dram_tensor("WOUT", [H, 1], f32, kind="ExternalInput")
    dB0 = nc.dram_tensor("B0", [H, 1], f32, kind="ExternalInput")
    dB1 = nc.dram_tensor("B1", [H, 1], f32, kind="ExternalInput")
    dB2 = nc.dram_tensor("B2", [H, 1], f32, kind="ExternalInput")
    dOUT = nc.dram_tensor("OUT", [NLOC, 1], f32, kind="ExternalOutput")

    with tile.TileContext(nc) as tc, \
         tc.tile_pool(name="sb", bufs=1) as sb, \
         tc.tile_pool(name="wk", bufs=1) as wk, \
         tc.tile_pool(name="ps", bufs=2, space=bass.MemorySpace.PSUM) as ps:

        # ---- loads (SWDGE) ----
        ldXT = sb.tile([F, NLOC], f32, name="ldXT")
        nc.gpsimd.dma_start(ldXT, dXT[:])
        ldYT = sb.tile([R, N], f32, name="ldYT")
        nc.gpsimd.dma_start(ldYT, dYT[:])
        ldUT = sb.tile([R, NLOC], f32, name="ldUT")
        nc.gpsimd.dma_start(ldUT, dUT[:])
        ldW0F = sb.tile([F, H], f32, name="ldW0F")
        nc.gpsimd.dma_start(ldW0F, dW0F[:])
        ldW0R = sb.tile([R, H], f32, name="ldW0R")
        nc.gpsimd.dma_start(ldW0R, dW0R[:])
        ldW1 = sb.tile([H, H], f32, name="ldW1")
        nc.gpsimd.dma_start(ldW1, dW1[:])
        ldW2 = sb.tile([H, H], f32, name="ldW2")
        nc.gpsimd.dma_start(ldW2, dW2[:])
        ldWO = sb.tile([H, 1], f32, name="ldWO")
        nc.gpsimd.dma_start(ldWO, dWO[:])
        ldB0 = sb.tile([H, 1], f32, name="ldB0")
        nc.gpsimd.dma_start(ldB0, dB0[:])
        ldB1 = sb.tile([H, 1], f32, name="ldB1")
        nc.gpsimd.dma_start(ldB1, dB1[:])
        ldB2 = sb.tile([H, 1], f32, name="ldB2")
        nc.gpsimd.dma_start(ldB2, dB2[:])

        # f32r roundings for the big matmuls (DVE converts on copy)
        sW1 = sb.tile([H, H], mmdt, name="sW1")
        nc.vector.tensor_copy(sW1, ldW1)
        sW2 = sb.tile([H, H], mmdt, name="sW2")
        nc.vector.tensor_copy(sW2, ldW2)
        sWO = sb.tile([H, 1], mmdt, name="sWO")
        nc.vector.tensor_copy(sWO, ldWO)

        # ---- preamble (plain f32 matmuls; small) ----
        hXTp = ps.tile([H, NLOC], f32, tag="mm", name="hXTp")
        nc.tensor.matmul(hXTp, ldW0F, ldXT, start=True, stop=True)
        hXT = sb.tile([H, NLOC], f32, name="hXT")
        nc.vector.tensor_copy(hXT, hXTp)

        hYp = ps.tile([H, N], f32, tag="mm", name="hYp")
        for k in range(2):
            nc.tensor.matmul(hYp[:, k * 512:(k + 1) * 512], ldW0R,
                             ldYT[:, k * 512:(k + 1) * 512],
                             start=True, stop=True)
        hYb0 = sb.tile([H, N], f32, name="hYb0")
        nc.scalar.activation(hYb0, hYp, AF.Identity, bias=ldB0, scale=1.0)

        costp = ps.tile([H, N], f32, tag="mm", name="costp")
        for k in range(2):
            nc.tensor.matmul(costp[:, k * 512:(k + 1) * 512], ldUT,
                             ldYT[:, k * 512:(k + 1) * 512],
                             start=True, stop=True)
        cost = sb.tile([H, N], f32, name="cost")
        nc.scalar.copy(cost, costp)

        psi = sb.tile([H, N], f32, name="psi")

        CW = AB * N  # columns per group

        def preadd(g, tgt):
            for k in range(AB):
                i = g * AB + k
                nc.vector.tensor_scalar_add(
                    tgt[:, k * N:(k + 1) * N], hYb0, hXT[:, i:i + 1])

        pcols = {}
        a2s = {}

        def mlp_l12(gg, a0t, rp=""):
            e1t = wk.tile([H, CW], f32, tag="e1", bufs=1, name=f"{rp}e1_{gg}")
            for s in range(2):
                l1t = ps.tile([H, 2048], f32, tag="mm", name=f"{rp}l1_{gg}_{s}")
                for k in range(4):
                    nc.tensor.matmul(
                        l1t[:, k * 512:(k + 1) * 512], sW1,
                        a0t[:, s * 2048 + k * 512: s * 2048 + (k + 1) * 512],
                        start=True, stop=True)
                nc.scalar.activation(e1t[:, s * 2048:(s + 1) * 2048], l1t,
                                     AF.Exp, bias=ldB1, scale=1.0)
            a1t = wk.tile([H, CW], mmdt, tag="a1", bufs=1, name=f"{rp}a1_{gg}")
            if True:
                # late groups: halve ln1 so l2(s0) unblocks sooner
                for s in range(2):
                    nc.scalar.activation(a1t[:, s * 2048:(s + 1) * 2048],
                                         e1t[:, s * 2048:(s + 1) * 2048],
                                         AF.Ln, bias=1.0, scale=1.0)
            else:
                nc.scalar.activation(a1t, e1t, AF.Ln, bias=1.0, scale=1.0)

            e2t = wk.tile([H, CW], f32, tag="e2", bufs=1, name=f"{rp}e2_{gg}")
            for s in range(2):
                l2t = ps.tile([H, 2048], f32, tag="mm", name=f"{rp}l2_{gg}_{s}")
                for k in range(4):
                    nc.tensor.matmul(
                        l2t[:, k * 512:(k + 1) * 512], sW2,
                        a1t[:, s * 2048 + k * 512: s * 2048 + (k + 1) * 512],
                        start=True, stop=True)
                nc.scalar.activation(e2t[:, s * 2048:(s + 1) * 2048], l2t,
                                     AF.Exp, bias=ldB2, scale=1.0)
            a2t = wk.tile([H, CW], mmdt, tag="a2", bufs=2, name=f"{rp}a2_{gg}")
            nc.scalar.activation(a2t, e2t, AF.Ln, bias=1.0, scale=1.0)
            a2s[gg] = a2t

        def psi_mms(gg, rp=""):
            a2t = a2s.pop(gg)
            for s in range(2):
                pr = ps.tile([1, 2048], f32, tag="mm",
                             name=f"{rp}psirow_{gg}_{s}")
                for k in range(4):
                    nc.tensor.matmul(
                        pr[0:1, k * 512:(k + 1) * 512], sWO[:, 0:1],
                        a2t[:, s * 2048 + k * 512: s * 2048 + (k + 1) * 512],
                        start=True, stop=True)
                pcols[(gg, s)] = pr

        def drains(gg, rp=""):
            for s in range(2):
                iA = gg * AB + 2 * s
                pr = pcols.pop((gg, s))
                row = wk.tile([1, 2048], f32, tag="rows", bufs=2,
                              name=f"{rp}row_{gg}_{s}")
                nc.vector.tensor_copy(row, pr)
                nc.sync.dma_start(psi[iA:iA + 1, :], row[0:1, 0:1024])
                nc.sync.dma_start(psi[iA + 1:iA + 2, :], row[0:1, 1024:2048])

        for rep in range(repeat):
            rp = f"r{rep}_"
            pre0_t = {}
            pre0_t[0] = wk.tile([H, CW], f32, tag="pre0", bufs=2,
                                name=rp + "pre0_0")
            preadd(0, pre0_t[0])
            for g in range(NGL):
                p0 = pre0_t.pop(g)
                nc.scalar.activation(p0, p0, AF.Exp)  # in place
                a0t = wk.tile([H, CW], mmdt, tag="a0", bufs=2,
                              name=f"{rp}a0_{g}")
                nc.scalar.activation(a0t, p0, AF.Ln, bias=1.0, scale=1.0)
                if g >= 3:
                    drains(g - 3, rp)
                if g + 1 < NGL:
                    pre0_t[g + 1] = wk.tile([H, CW], f32, tag="pre0", bufs=2,
                                            name=f"{rp}pre0_{g + 1}")
                    preadd(g + 1, pre0_t[g + 1])
                if g >= 1:
                    mlp_l12(g - 1, a0_prev, rp)
                if g >= 2:
                    psi_mms(g - 2, rp)
                a0_prev = a0t

            mlp_l12(NGL - 1, a0_prev, rp)

        # ---- tail: stabilized logmeanexp over j; split by partition range
        # so rows finished mid-loop overlap with the last groups ----
        # rows complete before the epilogue drains, 32-aligned (engine
        # partition offsets must be multiples of 32)
        PA = max(32, ((NGL - 4) * AB) // 32 * 32)
        slack = sb.tile([H, N], f32, name="slack")
        rowmax = sb.tile([H, 1], f32, name="rowmax")
        negb = sb.tile([H, 1], f32, name="negb")
        et = sb.tile([H, N], f32, name="et")
        rowsum = sb.tile([H, 1], f32, name="rowsum")
        lns = sb.tile([H, 1], f32, name="lns")
        v1 = sb.tile([H, 1], f32, name="v1")
        v2 = sb.tile([H, 1], f32, name="v2")
        psie = sb.tile([H, 1], f32, name="psie")

        def tail_part(lo, hi):
            nc.vector.tensor_sub(slack[lo:hi, :], cost[lo:hi, :],
                                 psi[lo:hi, :])
            nc.vector.reduce_max(rowmax[lo:hi], slack[lo:hi, :], axis=X_)
            nc.vector.tensor_scalar_mul(negb[lo:hi], rowmax[lo:hi], -10.0)
            nc.scalar.activation(et[lo:hi, :], slack[lo:hi, :], AF.Exp,
                                 bias=negb[lo:hi], scale=10.0,
                                 accum_out=rowsum[lo:hi])
            nc.scalar.activation(lns[lo:hi], rowsum[lo:hi], AF.Ln)
            nc.vector.tensor_scalar_mul(v1[lo:hi], lns[lo:hi], EPS)
            nc.vector.tensor_scalar_add(v2[lo:hi], v1[lo:hi],
                                        -EPS * math.log(float(N)))
            nc.vector.tensor_add(psie[lo:hi], v2[lo:hi], rowmax[lo:hi])

        rp = f"r{repeat - 1}_"
        psi_mms(NGL - 2, rp)
        drains(NGL - 3, rp)
        drains(NGL - 2, rp)
        tail_part(0, PA)
        psi_mms(NGL - 1, rp)
        drains(NGL - 1, rp)
        tail_part(PA, H)

        nc.sync.dma_start(dOUT[:], psie)

    _split_waits(nc, mybir)
    return nc


def _get_nc():
    import os
    variant = os.environ.get("KERNEL_VARIANT", "f32r")
    repeat = int(os.environ.get("KERNEL_REPEAT", "1"))
    ng = os.environ.get("KERNEL_NG")
    ng = int(ng) if ng else None
    key = ("nc", variant, repeat, ng)
    if key not in _cached:
        _cached[key] = _build(variant, repeat, ng)
    return _cached[key]


def kernel(**inputs):
    from concourse.bass_utils import run_bass_kernel_spmd

    X = np.asarray(inputs["X"], np.float32)
    U = np.asarray(inputs["U"], np.float32)
    Y = np.asarray(inputs["Y"], np.float32)
    W0 = np.asarray(inputs["W0"], np.float32)
    b0 = np.asarray(inputs["b0"], np.float32)
    W1 = np.asarray(inputs["W1"], np.float32)
    b1 = np.asarray(inputs["b1"], np.float32)
    W2 = np.asarray(inputs["W2"], np.float32)
    b2 = np.asarray(inputs["b2"], np.float32)
    Wout = np.asarray(inputs["Wout"], np.float32)
    bout = np.asarray(inputs["bout"], np.float32)

    YT = np.ascontiguousarray(Y.T)
    shared = {
        "YT": YT,
        "W0F": np.ascontiguousarray(W0[:F]),
        "W0R": np.ascontiguousarray(W0[F:]),
        "W1": np.ascontiguousarray(W1),
        "W2": np.ascontiguousarray(W2),
        "WOUT": np.ascontiguousarray(Wout),
        "B0": np.ascontiguousarray(b0[:, None]),
        "B1": np.ascontiguousarray(b1[:, None]),
        "B2": np.ascontiguousarray(b2[:, None]),
    }
    in_maps = []
    for c in range(NCORES):
        sl = slice(c * NLOC, (c + 1) * NLOC)
        m = dict(shared)
        m["XT"] = np.ascontiguousarray(X[sl].T)
        m["UT"] = np.ascontiguousarray(U[sl].T)
        in_maps.append(m)

    nc = _get_nc()
    res = run_bass_kernel_spmd(nc, in_maps, core_ids=list(range(NCORES)))
    _cached["last_res"] = res
    out = np.concatenate([res.results[c]["OUT"] for c in range(NCORES)], axis=0)
    return (out - bout[0]).astype(np.float32)


# revision 30
# speedup vs baseline: 1.3910x; 1.0066x over previous
"""Trainium2 Bass kernel for nn_EntropicOTQuantileRegression.

Math (reference):
    hX = X @ W0[:64]; hY = Y @ W0[64:]
    h  = sp(hX[i] + hY[j] + b0); h = sp(h@W1+b1); h = sp(h@W2+b2)
    psi[i,j] = (h @ Wout + bout); cost = U @ Y.T
    out[i] = eps*logmeanexp((cost[i,:]-psi[i,:])/eps)        (eps=0.1)

Sharding: 8 cores, data parallel over rows of X (128 rows each); Y/U-rows/
weights replicated or sharded accordingly; no collectives.

On-core layout: hidden dim (128) on partitions, pair columns on the free
axis. Per group of 4 local i's (4096 pair-columns):
    pre0 = hYT_b0 + hXT[:,i]          (DVE, per-i broadcast add)
    a0   = Ln(Exp(pre0)+1)            (ACT; softplus = exp+ln, no native op;
                                       Exp runs in place on pre0)
    l1   = W1.T @ a0 -> PSUM          (PE, f32r, 2x[128,2048] psum slots)
    a1   = Ln(Exp(l1 + b1)+1)         (ACT; ln halved per subgroup so l2
                                       unblocks sooner)
    l2, a2 likewise
    psi  = Wout.T @ a2 -> psum row    (PE, M=1 row matmuls, deferred one
                                       group; DVE drains the [1,2048] rows,
                                       SBUF->SBUF DMA scatters them to the
                                       per-i partitions of psi)
Tail: stabilized logmeanexp over j (rowmax via DVE reduce, Exp with
accum_out row sums, Ln), split at partition 96 so most rows overlap the
last groups. bout folded in on host. ACT (ScalarE) is the bottleneck
(~730 us busy of ~754 us total); the schedule keeps it ~95% utilized.

Toolchain constraints handled here: walrus accepts at most ONE semaphore
wait per instruction (_split_waits hoists extras onto injected NoOps; the
16-wait kernel-tail drain is split across engines), f32r producers must
write f32r-typed tiles, engine APs need 32-aligned partition offsets, and
DMA cannot touch PSUM.
"""
import math

import numpy as np

N = 1024
F = 64
R = 8
H = 128
NCORES = 8
NLOC = N // NCORES          # 128 local i rows per core
AB = 4                      # i's per act0 group
NG = NLOC // AB             # 32 groups (default; override via KERNEL_NG)
EPS = 0.1

_cached = {}


def _patch_drain():
    import concourse.tile as tile
    import concourse.mybir as mybir
    from concourse.vector_clock import ScopedClock

    if getattr(tile.TileContext, "_ant_drain_patched", False):
        return

    def _drain_and_barrier(self, tick_clock, wait_clock):
        nc = self.nc
        d0 = nc.sync.drain()
        wait_clock.add_sem_waits(
            d0.ins, ScopedClock({None: tick_clock.global_clock})
        )
        si = d0.ins.sync_info
        if si is not None and si.on_wait and len(si.on_wait) > 1:
            rest = list(si.on_wait[1:])
            d0.ins.sync_info = mybir.SyncInfo(
                on_wait=[si.on_wait[0]], on_update=list(si.on_update or [])
            )
            # spread the remaining waits across engines so they wait in
            # parallel; the all-engine barrier below joins them.
            engs = [nc.vector, nc.scalar, nc.tensor, nc.gpsimd, nc.sync]
            for idx, w in enumerate(rest):
                e = engs[idx % len(engs)]
                d = e.drain()
                d.ins.sync_info = mybir.SyncInfo(on_wait=[w], on_update=[])
        nc.all_engine_barrier()
        assert self.sems is not None
        popped = nc._tile_sem_poison_stack.pop()
        assert popped is self._sem_poison
        nc.clear_and_free_semaphores(list(self.sems.allocated().values()))
        nc.all_engine_barrier()

    tile.TileContext._drain_and_barrier = _drain_and_barrier
    tile.TileContext._ant_drain_patched = True


def _split_waits(nc, mybir):
    """Walrus in this toolchain accepts at most one semaphore wait per
    instruction; hoist extra waits onto injected same-engine NoOps."""
    n = 0
    for f in nc.m.functions:
        for bb in f.blocks:
            out = []
            for inst in bb.instructions:
                si = getattr(inst, "sync_info", None)
                if si is not None and si.on_wait and len(si.on_wait) > 1:
                    waits = list(si.on_wait)
                    for w in waits[:-1]:
                        out.append(mybir.InstNoOp(
                            name=f"antw-{nc.next_id()}",
                            engine=inst.engine,
                            sync_info=mybir.SyncInfo(on_wait=[w], on_update=[]),
                        ))
                    inst.sync_info = mybir.SyncInfo(
                        on_wait=[waits[-1]],
                        on_update=list(si.on_update or []),
                    )
                    n += 1
                out.append(inst)
            bb.instructions = out
    return n


def _build(variant="f32", repeat=1, ng=None):
    import concourse.bass as bass
    import concourse.tile as tile
    from concourse import mybir

    _patch_drain()
    NGL = NG if ng is None else ng

    f32 = mybir.dt.float32
    f32r = mybir.dt.float32r
    mmdt = f32r if variant == "f32r" else f32
    AF = mybir.ActivationFunctionType
    X_ = mybir.AxisListType.X

    nc = bass.Bass()
    dXT = nc.dram_tensor("XT", [F, NLOC], f32, kind="ExternalInput")
    dYT = nc.dram_tensor("YT", [R, N], f32, kind="ExternalInput")
    dUT = nc.dram_tensor("UT", [R, NLOC], f32, kind="ExternalInput")
    dW0F = nc.dram_tensor("W0F", [F, H], f32, kind="ExternalInput")
    dW0R = nc.dram_tensor("W0R", [R, H], f32, kind="ExternalInput")
    dW1 = nc.dram_tensor("W1", [H, H], f32, kind="ExternalInput")
    dW2 = nc.dram_tensor("W2", [H, H], f32, kind="ExternalInput")
    dWO = nc.dram_tensor("WOUT", [H, 1], f32, kind="ExternalInput")
    dB0 = nc.dram_tensor("B0", [H, 1], f32, kind="ExternalInput")
    dB1 = nc.dram_tensor("B1", [H, 1], f32, kind="ExternalInput")
    dB2 = nc.dram_tensor("B2", [H, 1], f32, kind="ExternalInput")
    dOUT = nc.dram_tensor("OUT", [NLOC, 1], f32, kind="ExternalOutput")

    with tile.TileContext(nc) as tc, \
         tc.tile_pool(name="sb", bufs=1) as sb, \
         tc.tile_pool(name="wk", bufs=1) as wk, \
         tc.tile_pool(name="ps", bufs=2, space=bass.MemorySpace.PSUM) as ps:

        # ---- loads (SWDGE) ----
        ldXT = sb.tile([F, NLOC], f32, name="ldXT")
        nc.gpsimd.dma_start(ldXT, dXT[:])
        ldYT = sb.tile([R, N], f32, name="ldYT")
        nc.gpsimd.dma_start(ldYT, dYT[:])
        ldUT = sb.tile([R, NLOC], f32, name="ldUT")
        nc.gpsimd.dma_start(ldUT, dUT[:])
        ldW0F = sb.tile([F, H], f32, name="ldW0F")
        nc.gpsimd.dma_start(ldW0F, dW0F[:])
        ldW0R = sb.tile([R, H], f32, name="ldW0R")
        nc.gpsimd.dma_start(ldW0R, dW0R[:])
        ldW1 = sb.tile([H, H], f32, name="ldW1")
        nc.gpsimd.dma_start(ldW1, dW1[:])
        ldW2 = sb.tile([H, H], f32, name="ldW2")
        nc.gpsimd.dma_start(ldW2, dW2[:])
        ldWO = sb.tile([H, 1], f32, name="ldWO")
        nc.gpsimd.dma_start(ldWO, dWO[:])
        ldB0 = sb.tile([H, 1], f32, name="ldB0")
        nc.gpsimd.dma_start(ldB0, dB0[:])
        ldB1 = sb.tile([H, 1], f32, name="ldB1")
        nc.gpsimd.dma_start(ldB1, dB1[:])
        ldB2 = sb.tile([H, 1], f32, name="ldB2")
        nc.gpsimd.dma_start(ldB2, dB2[:])

        # f32r roundings for the big matmuls (DVE converts on copy)
        sW1 = sb.tile([H, H], mmdt, name="sW1")
        nc.vector.tensor_copy(sW1, ldW1)
        sW2 = sb.tile([H, H], mmdt, name="sW2")
        nc.vector.tensor_copy(sW2, ldW2)
        sWO = sb.tile([H, 1], mmdt, name="sWO")
        nc.vector.tensor_copy(sWO, ldWO)

        # ---- preamble (plain f32 matmuls; small) ----
        hXTp = ps.tile([H, NLOC], f32, tag="mm", name="hXTp")
        nc.tensor.matmul(hXTp, ldW0F, ldXT, start=True, stop=True)
        hXT = sb.tile([H, NLOC], f32, name="hXT")
        nc.vector.tensor_copy(hXT, hXTp)

        hYp = ps.tile([H, N], f32, tag="mm", name="hYp")
        for k in range(2):
            nc.tensor.matmul(hYp[:, k * 512:(k + 1) * 512], ldW0R,
                             ldYT[:, k * 512:(k + 1) * 512],
                             start=True, stop=True)
        hYb0 = sb.tile([H, N], f32, name="hYb0")
        nc.scalar.activation(hYb0, hYp, AF.Identity, bias=ldB0, scale=1.0)

        costp = ps.tile([H, N], f32, tag="mm", name="costp")
        for k in range(2):
            nc.tensor.matmul(costp[:, k * 512:(k + 1) * 512], ldUT,
                             ldYT[:, k * 512:(k + 1) * 512],
                             start=True, stop=True)
        cost = sb.tile([H, N], f32, name="cost")
        nc.scalar.copy(cost, costp)

        psi = sb.tile([H, N], f32, name="psi")

        CW = AB * N  # columns per group

        def preadd(g, tgt):
            for k in range(AB):
                i = g * AB + k
                nc.vector.tensor_scalar_add(
                    tgt[:, k * N:(k + 1) * N], hYb0, hXT[:, i:i + 1])

        pcols = {}
        a2s = {}

        def mlp_l12(gg, a0t, rp=""):
            e1t = wk.tile([H, CW], f32, tag="e1", bufs=1, name=f"{rp}e1_{gg}")
            for s in range(2):
                l1t = ps.tile([H, 2048], f32, tag="mm", name=f"{rp}l1_{gg}_{s}")
                for k in range(4):
                    nc.tensor.matmul(
                        l1t[:, k * 512:(k + 1) * 512], sW1,
                        a0t[:, s * 2048 + k * 512: s * 2048 + (k + 1) * 512],
                        start=True, stop=True)
                nc.scalar.activation(e1t[:, s * 2048:(s + 1) * 2048], l1t,
                                     AF.Exp, bias=ldB1, scale=1.0)
            a1t = wk.tile([H, CW], mmdt, tag="a1", bufs=1, name=f"{rp}a1_{gg}")
            if True:
                # late groups: halve ln1 so l2(s0) unblocks sooner
                for s in range(2):
                    nc.scalar.activation(a1t[:, s * 2048:(s + 1) * 2048],
                                         e1t[:, s * 2048:(s + 1) * 2048],
                                         AF.Ln, bias=1.0, scale=1.0)
            else:
                nc.scalar.activation(a1t, e1t, AF.Ln, bias=1.0, scale=1.0)

            e2t = wk.tile([H, CW], f32, tag="e2", bufs=1, name=f"{rp}e2_{gg}")
            for s in range(2):
                l2t = ps.tile([H, 2048], f32, tag="mm", name=f"{rp}l2_{gg}_{s}")
                for k in range(4):
                    nc.tensor.matmul(
                        l2t[:, k * 512:(k + 1) * 512], sW2,
                        a1t[:, s * 2048 + k * 512: s * 2048 + (k + 1) * 512],
                        start=True, stop=True)
                nc.scalar.activation(e2t[:, s * 2048:(s + 1) * 2048], l2t,
                                     AF.Exp, bias=ldB2, scale=1.0)
            a2t = wk.tile([H, CW], mmdt, tag="a2", bufs=2, name=f"{rp}a2_{gg}")
            if gg == NGL - 1:
                # last group: halve ln2 so the epilogue psi chain starts early
                for s in range(2):
                    nc.scalar.activation(a2t[:, s * 2048:(s + 1) * 2048],
                                         e2t[:, s * 2048:(s + 1) * 2048],
                                         AF.Ln, bias=1.0, scale=1.0)
            else:
                nc.scalar.activation(a2t, e2t, AF.Ln, bias=1.0, scale=1.0)
            a2s[gg] = a2t

        def psi_mms(gg, rp=""):
            a2t = a2s.pop(gg)
            for s in range(2):
                pr = ps.tile([1, 2048], f32, tag="mm",
                             name=f"{rp}psirow_{gg}_{s}")
                for k in range(4):
                    nc.tensor.matmul(
                        pr[0:1, k * 512:(k + 1) * 512], sWO[:, 0:1],
                        a2t[:, s * 2048 + k * 512: s * 2048 + (k + 1) * 512],
                        start=True, stop=True)
                pcols[(gg, s)] = pr

        def drains(gg, rp=""):
            for s in range(2):
                iA = gg * AB + 2 * s
                pr = pcols.pop((gg, s))
                row = wk.tile([1, 2048], f32, tag="rows", bufs=2,
                              name=f"{rp}row_{gg}_{s}")
                nc.vector.tensor_copy(row, pr)
                nc.sync.dma_start(psi[iA:iA + 1, :], row[0:1, 0:1024])
                nc.sync.dma_start(psi[iA + 1:iA + 2, :], row[0:1, 1024:2048])

        for rep in range(repeat):
            rp = f"r{rep}_"
            pre0_t = {}
            pre0_t[0] = wk.tile([H, CW], f32, tag="pre0", bufs=2,
                                name=rp + "pre0_0")
            preadd(0, pre0_t[0])
            for g in range(NGL):
                p0 = pre0_t.pop(g)
                nc.scalar.activation(p0, p0, AF.Exp)  # in place
                a0t = wk.tile([H, CW], mmdt, tag="a0", bufs=2,
                              name=f"{rp}a0_{g}")
                nc.scalar.activation(a0t, p0, AF.Ln, bias=1.0, scale=1.0)
                if g + 1 < NGL:
                    pre0_t[g + 1] = wk.tile([H, CW], f32, tag="pre0", bufs=2,
                                            name=f"{rp}pre0_{g + 1}")
                    preadd(g + 1, pre0_t[g + 1])
                if g >= 1:
                    mlp_l12(g - 1, a0_prev, rp)
                if g >= 2:
                    psi_mms(g - 2, rp)
                    drains(g - 2, rp)
                a0_prev = a0t

            mlp_l12(NGL - 1, a0_prev, rp)

        # ---- tail: stabilized logmeanexp over j; split by partition range
        # so rows finished mid-loop overlap with the last groups ----
        # rows complete before the epilogue drains, 32-aligned (engine
        # partition offsets must be multiples of 32)
        PA = max(32, ((NGL - 4) * AB) // 32 * 32)
        slack = sb.tile([H, N], f32, name="slack")
        rowmax = sb.tile([H, 1], f32, name="rowmax")
        negb = sb.tile([H, 1], f32, name="negb")
        et = sb.tile([H, N], f32, name="et")
        rowsum = sb.tile([H, 1], f32, name="rowsum")
        lns = sb.tile([H, 1], f32, name="lns")
        v1 = sb.tile([H, 1], f32, name="v1")
        v2 = sb.tile([H, 1], f32, name="v2")
        psie = sb.tile([H, 1], f32, name="psie")

        def tail_part(lo, hi):
            nc.vector.tensor_sub(slack[lo:hi, :], cost[lo:hi, :],
                                 psi[lo:hi, :])
            nc.vector.reduce_max(rowmax[lo:hi], slack[lo:hi, :], axis=X_)
            nc.vector.tensor_scalar_mul(negb[lo:hi], rowmax[lo:hi], -10.0)
            nc.scalar.activation(et[lo:hi, :], slack[lo:hi, :], AF.Exp,
                                 bias=negb[lo:hi], scale=10.0,
                                 accum_out=rowsum[lo:hi])
            nc.scalar.activation(lns[lo:hi], rowsum[lo:hi], AF.Ln)
            nc.vector.tensor_scalar_mul(v1[lo:hi], lns[lo:hi], EPS)
            nc.vector.tensor_scalar_add(v2[lo:hi], v1[lo:hi],
                                        -EPS * math.log(float(N)))
            nc.vector.tensor_add(psie[lo:hi], v2[lo:hi], rowmax[lo:hi])

        rp = f"r{repeat - 1}_"
        psi_mms(NGL - 2, rp)
        drains(NGL - 2, rp)
        tail_part(0, PA)
        psi_mms(NGL - 1, rp)
        drains(NGL - 1, rp)
        tail_part(PA, H)

        nc.sync.dma_start(dOUT[:], psie)

    _split_waits(nc, mybir)
    return nc


def _get_nc():
    import os
    variant = os.environ.get("KERNEL_VARIANT", "f32r")
    repeat = int(os.environ.get("KERNEL_REPEAT", "1"))
    ng = os.environ.get("KERNEL_NG")
    ng = int(ng) if ng else None
    key = ("nc", variant, repeat, ng)
    if key not in _cached:
        _cached[key] = _build(variant, repeat, ng)
    return _cached[key]


def kernel(**inputs):
    from concourse.bass_utils import run_bass_kernel_spmd

    X = np.asarray(inputs["X"], np.float32)
    U = np.asarray(inputs["U"], np.float32)
    Y = np.asarray(inputs["Y"], np.float32)
    W0 = np.asarray(inputs["W0"], np.float32)
    b0 = np.asarray(inputs["b0"], np.float32)
    W1 = np.asarray(inputs["W1"], np.float32)
    b1 = np.asarray(inputs["b1"], np.float32)
    W2 = np.asarray(inputs["W2"], np.float32)
    b2 = np.asarray(inputs["b2"], np.float32)
    Wout = np.asarray(inputs["Wout"], np.float32)
    bout = np.asarray(inputs["bout"], np.float32)

    YT = np.ascontiguousarray(Y.T)
    shared = {
        "YT": YT,
        "W0F": np.ascontiguousarray(W0[:F]),
        "W0R": np.ascontiguousarray(W0[F:]),
        "W1": np.ascontiguousarray(W1),
        "W2": np.ascontiguousarray(W2),
        "WOUT": np.ascontiguousarray(Wout),
        "B0": np.ascontiguousarray(b0[:, None]),
        "B1": np.ascontiguousarray(b1[:, None]),
        "B2": np.ascontiguousarray(b2[:, None]),
    }
    in_maps = []
    for c in range(NCORES):
        sl = slice(c * NLOC, (c + 1) * NLOC)
        m = dict(shared)
        m["XT"] = np.ascontiguousarray(X[sl].T)
        m["UT"] = np.ascontiguousarray(U[sl].T)
        in_maps.append(m)

    nc = _get_nc()
    res = run_bass_kernel_spmd(nc, in_maps, core_ids=list(range(NCORES)))
    _cached["last_res"] = res
    out = np.concatenate([res.results[c]["OUT"] for c in range(NCORES)], axis=0)
    return (out - bout[0]).astype(np.float32)
